# revision 1
# baseline (speedup 1.0000x reference)
"""Octahedral SHT on 8 NeuronCores (Bass/Tile).

Strategy: shard the 192 latitude rings across 8 cores (24 rings each). Each
ring's ragged DFT (nlon in 20..400) is cut into K=128 chunks, zero-padded;
the ring assignment is engineered so every core gets exactly 51 chunks
-> one uniform SPMD program. The per-ring Legendre weights are replicated
per chunk, which folds the intra-ring chunk reduction into phase 2.
Each core computes a partial [l, m, bev] coefficient tensor over its own
rings; the host sums the 8 partials and assembles the complex output.

Precision: fp32 matmuls on the PE are 4x slower, so each fp32 operand is
split hi/lo into two fp16 tensors (x = hi + lo, |lo| <= 2^-11 |x|).
fp16*fp16 products are exact in the fp32 PSUM accumulator, so accumulating
MMs (hi*hi + hi*lo + lo*hi) reproduce the fp32 product to ~2^-22.

Phase 1 (per chunk c): G[c][m, (r,bev)] = E2[c].T @ x[c]  (PE, 6 MMs N=128,
  psum partition dim = m so the flatten yields m-major G' rows)
Flatten: G'[row c] <- [m, (hi|lo)] bounced through DRAM: SBUF->DRAM writes
  run at full HBM rate (the direct SBUF->SBUF gather is wall-limited by
  single-partition write bandwidth), then G' loads back in m-quarters whose
  SBUF writes spread across all 51 partitions, pipelined with phase-2 MMs.
  Row layout m*512 + h*256 + r*128 + bev.
Phase 2 (per m): out[l, (r,bev)] = 3 MMs (K=51):
  pw_hi @ G'hi (start) ; pw_hi @ G'lo ; pw_lo @ G'hi (stop)
"""
import numpy as np

NLAT, LMAX, MMAX = 192, 128, 128
B, V = 2, 64
BF = B * V            # 128 fused batch (b*64+v)
NCORES = 8
CHUNK = 128
NCH = 51              # chunks per core
RINGS_PER_CORE = 24
MAX_NLON = 400
NPTS = 40320
GB = [0, 4, 7, 10, 13, 17, 20, 23, 26, 30, 33, 36, 39, 43, 46, 49, NCH]
MG = 2                      # m's per psum tile (1 PSUM bank)
OG = 8                      # m's per out DMA group
PWG = 16                    # m's per pw/G' load group


def _octa_nlon():
    half = NLAT // 2
    north = np.array([4 * (i + 1) + 16 for i in range(half)], dtype=np.int64)
    return np.concatenate([north, north[::-1]])


def _ring_assignment():
    nlon = _octa_nlon()
    v = np.ceil(nlon / CHUNK).astype(int)
    cores = [[] for _ in range(NCORES)]
    for cls in (1, 2, 3, 4):
        ids = np.where(v == cls)[0]
        ids = ids[np.argsort(-nlon[ids], kind="stable")]
        fwd = True
        for start in range(0, len(ids), NCORES):
            blk = ids[start:start + NCORES]
            order = range(NCORES) if fwd else range(NCORES - 1, -1, -1)
            for c, rid in zip(order, blk):
                cores[c].append(int(rid))
            fwd = not fwd
    return cores, nlon


def _split16(a):
    hi = a.astype(np.float16)
    lo = (a - hi.astype(np.float32)).astype(np.float16)
    return hi, lo


def _build_core_inputs(core_rings, nlon, offs, x, E_re, E_im, PwT):
    """x: [BF, npts] f32.  Returns:
    xe  [128 j, 51 c, 768] f16  cols: [x_hi | x_lo | Ehi_re | Ehi_im | Elo_re | Elo_im]
    pw  [51, 128 m, 256] f16    cols: [pw_hi 0:128 | pw_lo 128:256]
    """
    xpad = np.zeros((NCH, CHUNK, BF), np.float32)
    E2 = np.zeros((NCH, CHUNK, 2 * MMAX), np.float32)
    Pw2 = np.zeros((MMAX, NCH, LMAX), np.float32)
    c = 0
    for r in core_rings:
        nl = int(nlon[r])
        o = int(offs[r])
        for j0 in range(0, nl, CHUNK):
            jlen = min(CHUNK, nl - j0)
            xpad[c, :jlen, :] = x[:, o + j0:o + j0 + jlen].T
            elen = min(CHUNK, MAX_NLON - j0)
            if elen > 0:
                E2[c, :elen, 0:MMAX] = E_re[r, j0:j0 + elen, :]
                E2[c, :elen, MMAX:] = E_im[r, j0:j0 + elen, :]
            Pw2[:, c, :] = PwT[:, r, :]
            c += 1
    assert c == NCH
    xh, xl = _split16(xpad)
    eh, el = _split16(E2)
    # eh/el cols: [re 0:128 | im 128:256]
    xe = np.concatenate([xh, xl, eh[:, :, 0:128], eh[:, :, 128:256],
                         el[:, :, 0:128], el[:, :, 128:256]], axis=2)
    xe = np.ascontiguousarray(xe.transpose(1, 0, 2))  # [128 j, 51 c, 768]

    ph, pl = _split16(Pw2)                           # [m, c, l] each
    pw = np.zeros((NCH, MMAX, 2 * LMAX), np.float16)
    pw[:, :, 0:128] = ph.transpose(1, 0, 2)          # pw_hi
    pw[:, :, 128:256] = pl.transpose(1, 0, 2)        # pw_lo
    return xe, pw


def _build_bass():
    import concourse.bass as bass
    import concourse.mybir as mybir
    from concourse import bacc, tile

    dt = mybir.dt
    nc = bacc.Bacc()

    xe_d = nc.dram_tensor("xe", [CHUNK, NCH, 768], dt.float16,
                          kind="ExternalInput")
    pw_d = nc.dram_tensor("pw", [NCH, MMAX, 2 * LMAX], dt.float16,
                          kind="ExternalInput")
    outp_d = nc.dram_tensor("outp", [LMAX, MMAX, 2 * BF], dt.float32,
                            kind="ExternalOutput")
    gdram = nc.dram_tensor("gdram", [NCH, MMAX * 512], dt.float16)

    with tile.TileContext(nc) as tc:
        with (
            tc.tile_pool(name="xs", bufs=4) as xs_pool,
            tc.tile_pool(name="gt", bufs=4) as gt_pool,
            tc.tile_pool(name="gs", bufs=4) as gs_pool,
            tc.tile_pool(name="pws", bufs=4) as pw_pool,
            tc.tile_pool(name="os", bufs=2) as os_pool,
            tc.tile_pool(name="ps1", bufs=2, space="PSUM") as ps1,
            tc.tile_pool(name="ps2", bufs=3, space="PSUM") as ps2,
        ):
            # ---- phase 1: 51 chunks x 6 accumulating MMs ----
            NG = len(GB) - 1
            xg = {}
            for g in range(NG):
                n = GB[g + 1] - GB[g]
                t = xs_pool.tile([CHUNK, n * 768], dt.float16, tag="xg")
                # split by partition halves: 2 parallel DMA engines per group
                nc.sync.dma_start(out=t[0:64, :],
                                  in_=xe_d[0:64, GB[g]:GB[g + 1], :])
                nc.scalar.dma_start(out=t[64:128, :],
                                    in_=xe_d[64:128, GB[g]:GB[g + 1], :])
                xg[g] = t

            for c in range(NCH):
                g = next(i for i in range(NG) if GB[i] <= c < GB[i + 1])
                off = (c - GB[g]) * 768
                xe = xg[g]
                xh = xe[:, off + 0:off + 128]
                xl = xe[:, off + 128:off + 256]
                ehr = xe[:, off + 256:off + 384]
                ehi = xe[:, off + 384:off + 512]
                elr = xe[:, off + 512:off + 640]
                eli = xe[:, off + 640:off + 768]
                # psum [m, (re_bev | im_bev)]; 6 MMs, lhsT = E slices
                g_ps = ps1.tile([MMAX, 2 * BF], dt.float32, tag="g")
                re = g_ps[:, 0:128]
                im = g_ps[:, 128:256]
                nc.tensor.matmul(re, ehr, xh, start=True, stop=False)
                nc.tensor.matmul(re, ehr, xl, start=False, stop=False)
                nc.tensor.matmul(re, elr, xh, start=False, stop=True)
                nc.tensor.matmul(im, ehi, xh, start=True, stop=False)
                nc.tensor.matmul(im, ehi, xl, start=False, stop=False)
                nc.tensor.matmul(im, eli, xh, start=False, stop=True)
                # evacuate PSUM, splitting fp32 -> fp16 hi (ACT) + lo (DVE)
                g_hl = gt_pool.tile([MMAX, 512], dt.float16, tag="ghl")
                nc.scalar.copy(g_hl[:, 0:256], g_ps[:])
                nc.vector.tensor_sub(g_hl[:, 256:512], g_ps[:], g_hl[:, 0:256])
                # flatten to DRAM (full-rate HBM write, 1KB runs)
                nc.gpsimd.dma_start(out=gdram[c], in_=g_hl[:])

            # ---- phase 2: 128 m x 3 accumulating MMs ----
            for mg in range(0, MMAX, PWG):
                pwt = pw_pool.tile([NCH, PWG * 256], dt.float16, tag="pw")
                nc.sync.dma_start(out=pwt[:], in_=pw_d[:, mg:mg + PWG, :])
                gsb = gs_pool.tile([NCH, PWG * 512], dt.float16, tag="gq")
                # two parallel loads (8KB runs each); the first groups ride
                # the scalar queue, which is idle by the end of phase 1
                half = PWG * 256
                if mg < 32:
                    ea, eb = nc.scalar, nc.scalar
                else:
                    ea, eb = nc.sync, nc.gpsimd
                ea.dma_start(
                    out=gsb[:, 0:half],
                    in_=gdram[:, mg * 512:mg * 512 + half])
                eb.dma_start(
                    out=gsb[:, half:2 * half],
                    in_=gdram[:, mg * 512 + half:(mg + PWG) * 512])
                for m8 in range(mg, mg + PWG, OG):
                    lb = 96 if m8 >= 96 else (64 if m8 >= 64 else 0)
                    o_sb = os_pool.tile([LMAX, OG * 256], dt.float32, tag="ot")
                    for m4 in range(m8, m8 + OG, 2):
                        # two m's in separate psum banks, MMs interleaved so
                        # consecutive PE MMs hit independent accum chains
                        o_ps0 = ps2.tile([LMAX, 256], dt.float32, tag="o0")
                        o_ps1 = ps2.tile([LMAX, 256], dt.float32, tag="o1")
                        ops = [o_ps0, o_ps1]
                        for term in range(3):
                            for i, m in enumerate((m4, m4 + 1)):
                                mo = (m - mg) * 256
                                ml = (m - mg) * 512
                                pa = pwt[:, mo + 0:mo + 128]
                                pb = pwt[:, mo + 128:mo + 256]
                                lhs, rhs = (
                                    (pa, gsb[:, ml:ml + 256]),
                                    (pa, gsb[:, ml + 256:ml + 512]),
                                    (pb, gsb[:, ml:ml + 256]),
                                )[term]
                                nc.tensor.matmul(ops[i][:], lhs, rhs,
                                                 start=(term == 0),
                                                 stop=(term == 2))
                        # coeffs[l < m] == 0 structurally; evacuate only
                        # rows l >= lb (legal partition bases {0,64,96};
                        # output buffer is pre-zeroed)
                        for i in range(2):
                            oo = (m4 + i - m8) * 256
                            if (m4 // 2 + i) % 2 == 0:
                                nc.vector.tensor_copy(o_sb[lb:, oo:oo + 256],
                                                      ops[i][lb:, :])
                            else:
                                nc.scalar.copy(o_sb[lb:, oo:oo + 256],
                                               ops[i][lb:, :])
                    eng = nc.sync if (m8 // OG) % 2 == 0 else nc.gpsimd
                    eng.dma_start(out=outp_d[lb:, m8:m8 + OG, :],
                                  in_=o_sb[lb:, :])

    nc.compile()
    return nc


_CACHE = {}


def _get_compiled():
    if "nc" not in _CACHE:
        _CACHE["nc"] = _build_bass()
    return _CACHE["nc"]


def kernel(data, Pw, E_re, E_im, pad_idx):
    from concourse import bass_utils

    data = np.asarray(data)
    Pw = np.asarray(Pw, dtype=np.float32)
    E_re = np.asarray(E_re, dtype=np.float32)
    E_im = np.asarray(E_im, dtype=np.float32)

    cores, nlon = _ring_assignment()
    offs = np.concatenate([[0], np.cumsum(nlon)[:-1]])
    # 'b e p v -> (b e v) p'
    x = np.ascontiguousarray(
        np.transpose(data, (0, 1, 3, 2)).reshape(BF, NPTS).astype(np.float32))
    PwT = np.ascontiguousarray(np.transpose(Pw, (1, 2, 0)))  # [m, n, l]

    in_maps = []
    for c in range(NCORES):
        xe, pw = _build_core_inputs(cores[c], nlon, offs, x, E_re, E_im, PwT)
        in_maps.append({"xe": xe, "pw": pw})

    nc = _get_compiled()
    res = bass_utils.run_bass_kernel_spmd(nc, in_maps, list(range(NCORES)))
    _CACHE["last_results"] = res

    total = np.zeros((LMAX, MMAX, 2 * BF), np.float64)
    for r in res.results:
        total += r["outp"].astype(np.float64)
    total = total.astype(np.float32).reshape(LMAX, MMAX, 2, BF)
    cc = total[:, :, 0, :] + 1j * total[:, :, 1, :]  # [l, m, bev]
    cc = cc.reshape(LMAX, MMAX, B, V)
    out = np.transpose(cc, (2, 0, 1, 3))[:, None]    # [b, 1, l, m, v]
    return out.astype(np.complex64)



# revision 8
# speedup vs baseline: 2.6055x; 2.6055x over previous
"""Octahedral SHT on 8 NeuronCores (Bass/Tile) — v1 redesign.

Strategy (vs the hi/lo fp16 baseline): the rel-err gate is 2e-2, so plain
fp16 matmuls (err ~5e-4) suffice — 3x fewer PE matmuls and half the DMA.

Sharding: 204 north-hemisphere DFT chunks (ring j-blocks of 128) are dealt
across 8 cores as 26 uniform "slots" (4 pad slots); each slot also carries
the mirrored south ring's chunk, which shares the identical DFT matrix E
(nlon is symmetric north/south), halving E traffic and PE weight loads.

Phase 1 (per slot): psum[m, 512] = [re_n|re_s | im_n|im_s] via 2 matmuls
  (lhsT = E_re / E_im [j,128m], rhs = x pair [j, 256]). Evacuate fp32->fp16
  with 4 engine copies reordering to ring-major, DMA 2 rows to gdram
  ([52 rows, 128 m, 256] = per-ring [re|im] per m).
Phase 2 (per 16-m group): load gsb [128 rows, 8 pairs x 256]: rows 0:52 =
  m's [16g,16g+8), rows 64:116 = m's [16g+8,16g+16) (pair p=(m, m+8) shares
  the same triangle length L = 128-16g). Per pair, 2 matmuls with K=128:
  lhsT = G_re / G_im [128, 128 bev], rhs = block-diag pw [128, 2L]
  (rows 0:52 -> pw_m0 cols 0:L, rows 64:116 -> pw_m1 cols L:2L, zero rows
  kill the garbage lhsT rows). Computes only l >= 16*(m//16) (coeffs with
  l < m are structurally zero). Output fp16 [128 bev, 18432], host sums the
  8 partials and unpacks the triangle.
"""
import numpy as np

NLAT, LMAX, MMAX = 192, 128, 128
B, V = 2, 64
BF = B * V
NCORES = 8
CHUNK = 128
NSLOT = 26           # north-chunk slots per core (204 real + 4 pad)
NROWS = 2 * NSLOT    # 52 G rows per core
MAX_NLON = 400
NPTS = 40320

# phase-2 triangle: for m, keep l in [LB(m), 128)
def LB(m):
    return 16 * (m // 16)


def LLEN(m):
    return LMAX - LB(m)


# pair p = 8*grp + i -> (m0, m1) = (16*grp + i, 16*grp + i + 8); L = 128 - 16*grp
def PAIR_MS(p):
    grp, i = divmod(p, 8)
    return 16 * grp + i, 16 * grp + i + 8


PAIR_OFF = np.zeros(64, np.int64)
_off = 0
for _p in range(64):
    PAIR_OFF[_p] = _off
    _off += 4 * LLEN(PAIR_MS(_p)[0])
OUTW = int(_off)                      # 18432

PW_OFF = np.zeros(MMAX + 1, np.int64)
for _m in range(MMAX):
    PW_OFF[_m + 1] = PW_OFF[_m] + LLEN(_m)
PWW = int(PW_OFF[-1])                 # 9216

PD_OFF = np.zeros(64, np.int64)
_o = 0
for _p in range(64):
    PD_OFF[_p] = _o
    _o += 2 * LLEN(PAIR_MS(_p)[0])
PWDW = int(_o)                        # 9216


def _octa_nlon():
    half = NLAT // 2
    north = np.array([4 * (i + 1) + 16 for i in range(half)], dtype=np.int64)
    return np.concatenate([north, north[::-1]])


def _assign_slots():
    """Per-core list of NSLOT entries: (north_ring, j0) or None (pad).
    Rings may straddle core boundaries (each part applies the same pw row;
    the host-side sum over cores restores the full quadrature)."""
    nlon = _octa_nlon()
    north = list(range(96))
    north.sort(key=lambda r: (-int(np.ceil(nlon[r] / CHUNK)), r))
    chunks = []
    for r in north:
        for j0 in range(0, int(nlon[r]), CHUNK):
            chunks.append((r, j0))
    assert len(chunks) == 204
    cores = []
    idx = 0
    for c in range(NCORES):
        n = 26 if c < 4 else 25
        cores.append([chunks[idx + i] for i in range(n)] + [None] * (NSLOT - n))
        idx += n
    assert idx == 204
    return cores, nlon


def _build_core(slots, nlon, offs, x, E_re, E_im, PwT):
    xe = np.zeros((CHUNK, NSLOT, 2, BF), np.float16)
    ee = np.zeros((CHUNK, NSLOT, 2 * MMAX), np.float16)
    pwc = np.zeros((NROWS, PWW), np.float16)
    for s, slot in enumerate(slots):
        if slot is None:
            continue
        r, j0 = slot
        rs = NLAT - 1 - r
        nl = int(nlon[r])
        jlen = min(CHUNK, nl - j0)
        xe[:jlen, s, 0, :] = x[:, offs[r] + j0: offs[r] + j0 + jlen].T
        xe[:jlen, s, 1, :] = x[:, offs[rs] + j0: offs[rs] + j0 + jlen].T
        elen = min(CHUNK, MAX_NLON - j0)
        ee[:elen, s, 0:MMAX] = E_re[r, j0:j0 + elen, :]
        ee[:elen, s, MMAX:] = E_im[r, j0:j0 + elen, :]
        for k, ring in enumerate((r, rs)):
            row = 2 * s + k
            for m in range(MMAX):
                pwc[row, PW_OFF[m]:PW_OFF[m + 1]] = PwT[m, ring, LB(m):]
    pwd = np.zeros((128, PWDW), np.float16)
    for p in range(64):
        m0, m1 = PAIR_MS(p)
        L = LLEN(m0)
        o = PD_OFF[p]
        pwd[0:NROWS, o:o + L] = pwc[:, PW_OFF[m0]:PW_OFF[m0 + 1]]
        pwd[64:64 + NROWS, o + L:o + 2 * L] = pwc[:, PW_OFF[m1]:PW_OFF[m1 + 1]]
    return (np.ascontiguousarray(xe.reshape(CHUNK, NSLOT * 256)),
            np.ascontiguousarray(ee.reshape(CHUNK, NSLOT * 256)),
            pwd)


def _build_bass():
    import concourse.mybir as mybir
    from concourse import bacc, tile

    dt = mybir.dt
    nc = bacc.Bacc()

    xe_d = nc.dram_tensor("xe", [CHUNK, NSLOT * 256], dt.float16,
                          kind="ExternalInput")
    ee_d = nc.dram_tensor("ee", [CHUNK, NSLOT * 256], dt.float16,
                          kind="ExternalInput")
    pwd_d = nc.dram_tensor("pwd", [128, PWDW], dt.float16,
                           kind="ExternalInput")
    outp_d = nc.dram_tensor("outp", [128, OUTW], dt.float16,
                            kind="ExternalOutput")
    gdram = nc.dram_tensor("gdram", [NROWS, MMAX * 256], dt.float16)

    with tile.TileContext(nc) as tc:
        with (
            tc.tile_pool(name="inp", bufs=1) as in_pool,
            tc.tile_pool(name="gsl", bufs=4) as gsl_pool,
            tc.tile_pool(name="gsb", bufs=3) as gsb_pool,
            tc.tile_pool(name="osb", bufs=3) as osb_pool,
            tc.tile_pool(name="ps1", bufs=3, space="PSUM") as ps1,
            tc.tile_pool(name="ps2", bufs=4, space="PSUM") as ps2,
        ):
            xe = in_pool.tile([CHUNK, NSLOT * 256], dt.float16, tag="xe")
            ee = in_pool.tile([CHUNK, NSLOT * 256], dt.float16, tag="ee")
            pwd = in_pool.tile([128, PWDW], dt.float16, tag="pwd")

            # input loads: 4 slot-groups; xe on sync, ee on scalar (DMA
            # queues exist only on sync/scalar/gpsimd)
            GBL = [0, 7, 14, 20, 26]
            for g in range(4):
                c0, c1 = GBL[g] * 256, GBL[g + 1] * 256
                nc.sync.dma_start(out=xe[:, c0:c1], in_=xe_d[:, c0:c1])
                nc.scalar.dma_start(out=ee[:, c0:c1], in_=ee_d[:, c0:c1])
            # pw block-diag, needed only in phase 2
            nc.gpsimd.dma_start(out=pwd[:], in_=pwd_d[:])

            cp_engines = [nc.scalar, nc.vector]

            def cp(idx, out, in_):
                e = cp_engines[idx % 2]
                if e is nc.scalar:
                    e.copy(out, in_)
                else:
                    e.tensor_copy(out, in_)

            # ---- phase 1 ----
            ci = 0
            for s in range(NSLOT):
                g = ps1.tile([MMAX, 512], dt.float32, tag="g")
                rhs = xe[:, s * 256:(s + 1) * 256]
                nc.tensor.matmul(g[:, 0:256], ee[:, s * 256:s * 256 + 128],
                                 rhs, start=True, stop=True)
                nc.tensor.matmul(g[:, 256:512],
                                 ee[:, s * 256 + 128:(s + 1) * 256],
                                 rhs, start=True, stop=True)
                gsl = gsl_pool.tile([MMAX, 512], dt.float16, tag="gsl")
                # reorder [re_n|re_s|im_n|im_s] -> ring-major [re_n|im_n|re_s|im_s]
                cp(ci + 0, gsl[:, 0:128], g[:, 0:128])
                cp(ci + 1, gsl[:, 128:256], g[:, 256:384])
                cp(ci + 2, gsl[:, 256:384], g[:, 128:256])
                cp(ci + 3, gsl[:, 384:512], g[:, 384:512])
                ci += 4
                we = nc.sync if s % 2 == 0 else nc.scalar
                we.dma_start(out=gdram[2 * s], in_=gsl[:, 0:256])
                we.dma_start(out=gdram[2 * s + 1], in_=gsl[:, 256:512])

            # ---- phase 2 ----
            for grp in range(8):
                L = 128 - 16 * grp
                mg = 16 * grp
                gsb = gsb_pool.tile([128, 2048], dt.float16, tag="gsb")
                nc.sync.dma_start(out=gsb[0:52, :],
                                  in_=gdram[:, mg * 256:(mg + 8) * 256])
                nc.scalar.dma_start(out=gsb[64:116, :],
                                    in_=gdram[:, (mg + 8) * 256:(mg + 16) * 256])
                per_bank = max(1, 512 // (4 * L))
                wb = per_bank * 4 * L          # valid bank width (<= 512)
                po = None
                osb = None
                osb_banks = 0
                osb_col0 = 0
                nosb = 2 if grp < 6 else 1
                for i in range(8):
                    p = 8 * grp + i
                    q = i % per_bank
                    if q == 0:
                        po = ps2.tile([128, 512], dt.float32, tag="po")
                    c0 = q * 4 * L
                    do = int(PD_OFF[p])
                    prhs = pwd[:, do:do + 2 * L]
                    nc.tensor.matmul(po[:, c0:c0 + 2 * L],
                                     gsb[:, i * 256:i * 256 + 128],
                                     prhs, start=True, stop=True)
                    nc.tensor.matmul(po[:, c0 + 2 * L:c0 + 4 * L],
                                     gsb[:, i * 256 + 128:(i + 1) * 256],
                                     prhs, start=True, stop=True)
                    if q == per_bank - 1:
                        # bank full -> evacuate valid cols only
                        if osb is None:
                            osb = osb_pool.tile([128, wb * nosb], dt.float16,
                                                tag="osb")
                            osb_banks = 0
                            osb_col0 = int(PAIR_OFF[p - per_bank + 1])
                        cp(ci, osb[:, osb_banks * wb:(osb_banks + 1) * wb],
                           po[:, 0:wb])
                        ci += 1
                        osb_banks += 1
                        if osb_banks == nosb:
                            nc.gpsimd.dma_start(
                                out=outp_d[:, osb_col0:osb_col0 + wb * nosb],
                                in_=osb[:])
                            osb = None

    nc.compile()
    return nc


_CACHE = {}


def _get_compiled():
    if "nc" not in _CACHE:
        _CACHE["nc"] = _build_bass()
    return _CACHE["nc"]


def kernel(data, Pw, E_re, E_im, pad_idx):
    from concourse import bass_utils

    data = np.asarray(data)
    Pw = np.asarray(Pw, dtype=np.float32)
    E_re = np.asarray(E_re, dtype=np.float32)
    E_im = np.asarray(E_im, dtype=np.float32)

    cores, nlon = _assign_slots()
    offs = np.concatenate([[0], np.cumsum(nlon)[:-1]])
    x = np.ascontiguousarray(
        np.transpose(data, (0, 1, 3, 2)).reshape(BF, NPTS).astype(np.float32))
    PwT = np.ascontiguousarray(np.transpose(Pw, (1, 2, 0)))  # [m, n, l]

    in_maps = []
    for c in range(NCORES):
        xe, ee, pwd = _build_core(cores[c], nlon, offs, x, E_re, E_im, PwT)
        in_maps.append({"xe": xe, "ee": ee, "pwd": pwd})

    nc = _get_compiled()
    res = bass_utils.run_bass_kernel_spmd(nc, in_maps, list(range(NCORES)))
    _CACHE["last_results"] = res

    total = np.zeros((BF, OUTW), np.float64)
    for r in res.results:
        total += r["outp"].astype(np.float64)
    total = total.astype(np.float32)

    cc = np.zeros((LMAX, MMAX, BF), np.complex64)
    for p in range(64):
        m0, m1 = PAIR_MS(p)
        L = LLEN(m0)
        o = int(PAIR_OFF[p])
        blk = total[:, o:o + 4 * L]
        lb = LB(m0)
        cc[lb:, m0, :] = (blk[:, 0:L] + 1j * blk[:, 2 * L:3 * L]).T
        cc[lb:, m1, :] = (blk[:, L:2 * L] + 1j * blk[:, 3 * L:4 * L]).T
    cc = cc.reshape(LMAX, MMAX, B, V)
    out = np.transpose(cc, (2, 0, 1, 3))[:, None]
    return out.astype(np.complex64)


# revision 17
# speedup vs baseline: 2.9606x; 1.1363x over previous
"""Octahedral SHT on 8 NeuronCores (Bass/Tile) — v2.

v1 -> v2: per-ring PSUM accumulation (chunks of one ring accumulate on-chip
before the DRAM bounce) and 4-way m-packing in phase 2.

Sharding: 204 north DFT chunks -> 8 cores x 27 slots, organized as 12
ring-groups per core with the uniform size pattern [4,3,3,3,3,2,2,2,2,1,1,1]
(27 slots). Ring classes fit exactly: 4-groups take the 4 four-chunk rings +
4 two-chunk rings (padded), 3-groups the 32 three-chunk rings, 2-groups
28 two-chunk + 4 one-chunk rings, 1-groups 24 one-chunk rings. Each slot
also carries the mirrored south ring's chunk (identical DFT matrix E since
nlon is north/south symmetric), halving E traffic and PE weight loads.

Phase 1 (per group): psum[m, 512] = [re_n|re_s | im_n|im_s] accumulated
over the group's chunks (2 matmuls per slot, start on first chunk / stop on
last). Evacuate fp32->fp16 with 4 copies reordering to ring-major rows,
DMA 2 rows to gdram [24 rows, 128 m, 256] (row = ring: [re|im] per m).
Phase 2 (per 16-m group, per quad qi<4): the quad covers m = 16g+qi+4b for
bands b=0..3; gsb [128, 1024]: band b rows [32b, 32b+24) <- gdram m-columns
(contiguous). 2 matmuls per quad, K=128: lhsT = G_re/G_im [128, 128 bev],
rhs = block-diag pw [128, 4L] (band b rows -> pw_m(b) cols [bL,(b+1)L));
zero pw rows kill garbage lhsT rows. Only l >= 16*(m//16) is computed
(coeffs with l < m are structurally zero). Output fp16 [128 bev, 18432];
host sums the 8 partials and unpacks the triangle.
"""
import numpy as np

NLAT, LMAX, MMAX = 192, 128, 128
B, V = 2, 64
BF = B * V
NCORES = 8
CHUNK = 128
GSIZES = [4, 3, 3, 3, 3, 2, 2, 2, 2, 1, 1, 1]
NG = len(GSIZES)                  # 12 ring-groups per core
NSLOT = sum(GSIZES)               # 27
NROWS = 2 * NG                    # 24 G rows per core
GOFF = np.concatenate([[0], np.cumsum(GSIZES)]).astype(np.int64)
MAX_NLON = 400
NPTS = 40320


def LB(m):
    return 16 * (m // 16)


def LLEN(m):
    return LMAX - LB(m)


# quad q = 4*grp + qi -> m's { 16*grp + qi + 4*b : b in 0..3 }, L = 128-16*grp
QUAD_OFF = np.zeros(32, np.int64)
_o = 0
for _q in range(32):
    QUAD_OFF[_q] = _o
    _o += 8 * LLEN(16 * (_q // 4))
OUTW = int(_o)                    # 18432

QD_OFF = np.zeros(32, np.int64)
_o = 0
for _q in range(32):
    QD_OFF[_q] = _o
    _o += 4 * LLEN(16 * (_q // 4))
PWDW = int(_o)                    # 9216


def _octa_nlon():
    half = NLAT // 2
    north = np.array([4 * (i + 1) + 16 for i in range(half)], dtype=np.int64)
    return np.concatenate([north, north[::-1]])


def _assign_groups():
    """Per-core list of NG north rings (group g -> one ring, padded to
    GSIZES[g] chunk slots)."""
    nlon = _octa_nlon()
    nch = np.ceil(nlon[:96] / CHUNK).astype(int)
    cls = {c: sorted(np.where(nch == c)[0].tolist()) for c in (1, 2, 3, 4)}
    assert [len(cls[c]) for c in (1, 2, 3, 4)] == [28, 32, 32, 4]
    c1, c2, c3, c4 = cls[1][:], cls[2][:], cls[3][:], cls[4][:]
    cores = []
    for c in range(NCORES):
        g4 = c4.pop() if c < 4 else c2.pop()
        g3s = [c3.pop() for _ in range(4)]
        g2s = [c2.pop() for _ in range(4)] if c < 4 else \
              [c2.pop() for _ in range(3)] + [c1.pop()]
        g1s = [c1.pop() for _ in range(3)]
        cores.append([g4] + g3s + g2s + g1s)
    assert not c1 and not c2 and not c3 and not c4
    return cores, nlon


def _build_core(rings, nlon, offs, x, E_re, E_im, PwT):
    xe = np.zeros((CHUNK, NSLOT, 2, BF), np.float16)
    ee = np.zeros((CHUNK, NSLOT, 2 * MMAX), np.float16)
    pwc = np.zeros((NROWS, MMAX, LMAX), np.float32)
    for g in range(NG):
        r = rings[g]
        rs = NLAT - 1 - r
        nl = int(nlon[r])
        for t in range(GSIZES[g]):
            j0 = t * CHUNK
            if j0 >= nl:
                continue
            s = int(GOFF[g]) + t
            jlen = min(CHUNK, nl - j0)
            xe[:jlen, s, 0, :] = x[:, offs[r] + j0: offs[r] + j0 + jlen].T
            xe[:jlen, s, 1, :] = x[:, offs[rs] + j0: offs[rs] + j0 + jlen].T
            elen = min(CHUNK, MAX_NLON - j0)
            ee[:elen, s, 0:MMAX] = E_re[r, j0:j0 + elen, :]
            ee[:elen, s, MMAX:] = E_im[r, j0:j0 + elen, :]
        pwc[2 * g] = PwT[:, r, :]          # [m, l]
        pwc[2 * g + 1] = PwT[:, rs, :]
    pwd = np.zeros((4 * NROWS, PWDW), np.float16)
    for q in range(32):
        grp, qi = divmod(q, 4)
        L = LLEN(16 * grp)
        lb = 16 * grp
        o = int(QD_OFF[q])
        for b in range(4):
            m = 16 * grp + qi + 4 * b
            pwd[NROWS * b:NROWS * (b + 1), o + b * L:o + (b + 1) * L] = \
                pwc[:, m, lb:]
    return (np.ascontiguousarray(xe.reshape(CHUNK, NSLOT * 256)),
            np.ascontiguousarray(ee.reshape(CHUNK, NSLOT * 256)),
            pwd)


def _build_bass():
    import concourse.mybir as mybir
    from concourse import bacc, tile

    dt = mybir.dt
    nc = bacc.Bacc()

    xe_d = nc.dram_tensor("xe", [CHUNK, NSLOT * 256], dt.float16,
                          kind="ExternalInput")
    ee_d = nc.dram_tensor("ee", [CHUNK, NSLOT * 256], dt.float16,
                          kind="ExternalInput")
    pwd_d = nc.dram_tensor("pwd", [4 * NROWS, PWDW], dt.float16,
                           kind="ExternalInput")
    outp_d = nc.dram_tensor("outp", [128, OUTW], dt.float16,
                            kind="ExternalOutput")
    gdram = nc.dram_tensor("gdram", [NROWS, MMAX * 256], dt.float16)

    with tile.TileContext(nc) as tc:
        with (
            tc.tile_pool(name="inp", bufs=1) as in_pool,
            tc.tile_pool(name="gsl", bufs=4) as gsl_pool,
            tc.tile_pool(name="gsb", bufs=3) as gsb_pool,
            tc.tile_pool(name="osb", bufs=3) as osb_pool,
            tc.tile_pool(name="ps1a", bufs=2, space="PSUM") as ps1a,
            tc.tile_pool(name="ps1b", bufs=2, space="PSUM") as ps1b,
            tc.tile_pool(name="ps2", bufs=4, space="PSUM") as ps2,
        ):
            xe = in_pool.tile([CHUNK, NSLOT * 256], dt.float16, tag="xe")
            ee = in_pool.tile([CHUNK, NSLOT * 256], dt.float16, tag="ee")
            pwd = in_pool.tile([4 * NROWS, PWDW], dt.float16, tag="pwd")

            # loads: xe on sync, ee on gpsimd (scalar stays copy-only in
            # phase 1); pwd late on gpsimd (phase-2 input)
            GBL = [0, 7, 14, 21, 27]
            for g in range(4):
                c0, c1 = GBL[g] * 256, GBL[g + 1] * 256
                nc.sync.dma_start(out=xe[:, c0:c1], in_=xe_d[:, c0:c1])
                nc.gpsimd.dma_start(out=ee[:, c0:c1], in_=ee_d[:, c0:c1])
            nc.gpsimd.dma_start(out=pwd[:], in_=pwd_d[:])

            cp_engines = [nc.scalar, nc.vector]

            def cp(idx, out, in_):
                e = cp_engines[idx % 2]
                if e is nc.scalar:
                    e.copy(out, in_)
                else:
                    e.tensor_copy(out, in_)

            # ---- phase 1: 12 ring-groups, psum accumulation over chunks ----
            ci = 0
            for g in range(NG):
                sz = GSIZES[g]
                # separate banks for the re / im accumulation chains (one
                # psum zero-region cannot host two pending groups)
                gre = ps1a.tile([MMAX, 512], dt.float32, tag="gre")
                gim = ps1b.tile([MMAX, 512], dt.float32, tag="gim")
                for t in range(sz):
                    s = int(GOFF[g]) + t
                    rhs = xe[:, s * 256:(s + 1) * 256]
                    st, sp = (t == 0), (t == sz - 1)
                    nc.tensor.matmul(gre[:, 0:256],
                                     ee[:, s * 256:s * 256 + 128],
                                     rhs, start=st, stop=sp)
                    nc.tensor.matmul(gim[:, 0:256],
                                     ee[:, s * 256 + 128:(s + 1) * 256],
                                     rhs, start=st, stop=sp)
                gsl = gsl_pool.tile([MMAX, 512], dt.float16, tag="gsl")
                # [re_n|re_s] + [im_n|im_s] -> ring-major [re_n|im_n|re_s|im_s]
                cp(ci + 0, gsl[:, 0:128], gre[:, 0:128])
                cp(ci + 1, gsl[:, 128:256], gim[:, 0:128])
                cp(ci + 2, gsl[:, 256:384], gre[:, 128:256])
                cp(ci + 3, gsl[:, 384:512], gim[:, 128:256])
                ci += 4
                we = nc.sync if g % 2 == 0 else nc.gpsimd
                we.dma_start(out=gdram[2 * g], in_=gsl[:, 0:256])
                we.dma_start(out=gdram[2 * g + 1], in_=gsl[:, 256:512])

            # ---- phase 2: 8 m-groups x 4 quads ----
            oq = 0   # outp write queue rotation
            oqs = [nc.gpsimd, nc.scalar, nc.sync]
            for grp in range(8):
                L = 128 - 16 * grp
                mg = 16 * grp
                # bands packed contiguously: row 24*b + r <- (ring r, band b)
                gsb = gsb_pool.tile([4 * NROWS, 1024], dt.float16, tag="gsb")
                src = gdram[:, mg * 256:(mg + 16) * 256]
                src = src.rearrange("r (b c) -> r b c", b=4).transpose([1, 0, 2])
                nc.sync.dma_start(out=gsb[:], in_=src)
                # psum packing: quads per bank (8L f32 per quad)
                per_bank = max(1, 512 // (8 * L))   # 0 means quad spans banks
                po = None
                osb = None
                osb_banks = 0
                osb_col0 = 0
                if L > 64:
                    # quad re / im each need their own psum bank (4L > 256)
                    for qi in range(4):
                        q = 4 * grp + qi
                        do = int(QD_OFF[q])
                        prhs = pwd[:, do:do + 4 * L]
                        col0 = int(QUAD_OFF[q])
                        osb = osb_pool.tile([128, 8 * L], dt.float16,
                                            tag="osb")
                        for h in range(2):   # 0: re, 1: im
                            po = ps2.tile([128, 512], dt.float32, tag="po")
                            nc.tensor.matmul(
                                po[:, 0:4 * L],
                                gsb[:, qi * 256 + h * 128:qi * 256 + h * 128 + 128],
                                prhs, start=True, stop=True)
                            cp(ci, osb[:, h * 4 * L:(h + 1) * 4 * L],
                               po[:, 0:4 * L])
                            ci += 1
                        oqs[oq % 3].dma_start(
                            out=outp_d[:, col0:col0 + 8 * L], in_=osb[:])
                        oq += 1
                else:
                    # whole quads (re+im = 8L <= 512) pack into banks
                    nosb = 2 if grp < 6 else 1
                    wb = per_bank * 8 * L
                    for qi in range(4):
                        q = 4 * grp + qi
                        do = int(QD_OFF[q])
                        prhs = pwd[:, do:do + 4 * L]
                        bq = qi % per_bank
                        if bq == 0:
                            po = ps2.tile([128, 512], dt.float32, tag="po")
                        c0 = bq * 8 * L
                        nc.tensor.matmul(po[:, c0:c0 + 4 * L],
                                         gsb[:, qi * 256:qi * 256 + 128],
                                         prhs, start=True, stop=True)
                        nc.tensor.matmul(po[:, c0 + 4 * L:c0 + 8 * L],
                                         gsb[:, qi * 256 + 128:(qi + 1) * 256],
                                         prhs, start=True, stop=True)
                        if bq == per_bank - 1:
                            if osb is None:
                                osb = osb_pool.tile([128, wb * nosb],
                                                    dt.float16, tag="osb")
                                osb_banks = 0
                                osb_col0 = int(QUAD_OFF[q - per_bank + 1])
                            cp(ci, osb[:, osb_banks * wb:(osb_banks + 1) * wb],
                               po[:, 0:wb])
                            ci += 1
                            osb_banks += 1
                            if osb_banks == nosb:
                                oqs[oq % 3].dma_start(
                                    out=outp_d[:, osb_col0:osb_col0 + wb * nosb],
                                    in_=osb[:])
                                oq += 1
                                osb = None

    nc.compile()
    return nc


_CACHE = {}


def _get_compiled():
    if "nc" not in _CACHE:
        _CACHE["nc"] = _build_bass()
    return _CACHE["nc"]


def kernel(data, Pw, E_re, E_im, pad_idx):
    from concourse import bass_utils

    data = np.asarray(data)
    Pw = np.asarray(Pw, dtype=np.float32)
    E_re = np.asarray(E_re, dtype=np.float32)
    E_im = np.asarray(E_im, dtype=np.float32)

    cores, nlon = _assign_groups()
    offs = np.concatenate([[0], np.cumsum(nlon)[:-1]])
    x = np.ascontiguousarray(
        np.transpose(data, (0, 1, 3, 2)).reshape(BF, NPTS).astype(np.float32))
    PwT = np.ascontiguousarray(np.transpose(Pw, (1, 2, 0)))  # [m, n, l]

    in_maps = []
    for c in range(NCORES):
        xe, ee, pwd = _build_core(cores[c], nlon, offs, x, E_re, E_im, PwT)
        in_maps.append({"xe": xe, "ee": ee, "pwd": pwd})

    nc = _get_compiled()
    res = bass_utils.run_bass_kernel_spmd(nc, in_maps, list(range(NCORES)))
    _CACHE["last_results"] = res

    total = np.zeros((BF, OUTW), np.float64)
    for r in res.results:
        total += r["outp"].astype(np.float64)
    total = total.astype(np.float32)

    cc = np.zeros((LMAX, MMAX, BF), np.complex64)
    for q in range(32):
        grp, qi = divmod(q, 4)
        L = LLEN(16 * grp)
        lb = 16 * grp
        o = int(QUAD_OFF[q])
        for b in range(4):
            m = 16 * grp + qi + 4 * b
            re = total[:, o + b * L:o + (b + 1) * L]
            im = total[:, o + 4 * L + b * L:o + 4 * L + (b + 1) * L]
            cc[lb:, m, :] = (re + 1j * im).T
    cc = cc.reshape(LMAX, MMAX, B, V)
    out = np.transpose(cc, (2, 0, 1, 3))[:, None]
    return out.astype(np.complex64)


# revision 20
# speedup vs baseline: 3.4593x; 1.1684x over previous
"""Octahedral SHT on 8 NeuronCores (Bass/Tile) — v2.

v1 -> v2: per-ring PSUM accumulation (chunks of one ring accumulate on-chip
before the DRAM bounce) and 4-way m-packing in phase 2.

Sharding: 204 north DFT chunks -> 8 cores x 27 slots, organized as 12
ring-groups per core with the uniform size pattern [4,3,3,3,3,2,2,2,2,1,1,1]
(27 slots). Ring classes fit exactly: 4-groups take the 4 four-chunk rings +
4 two-chunk rings (padded), 3-groups the 32 three-chunk rings, 2-groups
28 two-chunk + 4 one-chunk rings, 1-groups 24 one-chunk rings. Each slot
also carries the mirrored south ring's chunk (identical DFT matrix E since
nlon is north/south symmetric), halving E traffic and PE weight loads.

Phase 1 (per group): psum[m, 512] = [re_n|re_s | im_n|im_s] accumulated
over the group's chunks (2 matmuls per slot, start on first chunk / stop on
last). Evacuate fp32->fp16 with 4 copies reordering to ring-major rows,
DMA 2 rows to gdram [24 rows, 128 m, 256] (row = ring: [re|im] per m).
Phase 2 (per 16-m group, per quad qi<4): the quad covers m = 16g+qi+4b for
bands b=0..3; gsb [128, 1024]: band b rows [32b, 32b+24) <- gdram m-columns
(contiguous). 2 matmuls per quad, K=128: lhsT = G_re/G_im [128, 128 bev],
rhs = block-diag pw [128, 4L] (band b rows -> pw_m(b) cols [bL,(b+1)L));
zero pw rows kill garbage lhsT rows. Only l >= 16*(m//16) is computed
(coeffs with l < m are structurally zero). Output fp16 [128 bev, 18432];
host sums the 8 partials and unpacks the triangle.
"""
import numpy as np

NLAT, LMAX, MMAX = 192, 128, 128
B, V = 2, 64
BF = B * V
NCORES = 8
CHUNK = 128
GSIZES = [4, 3, 3, 3, 3, 2, 2, 2, 2, 1, 1, 1]
NG = len(GSIZES)                  # 12 ring-groups per core
NSLOT = sum(GSIZES)               # 27
NROWS = 2 * NG                    # 24 G rows per core
GOFF = np.concatenate([[0], np.cumsum(GSIZES)]).astype(np.int64)
MAX_NLON = 400
NPTS = 40320


def LB(m):
    return 16 * (m // 16)


def LLEN(m):
    return LMAX - LB(m)


# quad q = 4*grp + qi -> m's { 16*grp + qi + 4*b : b in 0..3 }, L = 128-16*grp
QUAD_OFF = np.zeros(32, np.int64)
_o = 0
for _q in range(32):
    QUAD_OFF[_q] = _o
    _o += 8 * LLEN(16 * (_q // 4))
OUTW = int(_o)                    # 18432

QD_OFF = np.zeros(32, np.int64)
_o = 0
for _q in range(32):
    QD_OFF[_q] = _o
    _o += 4 * LLEN(16 * (_q // 4))
PWDW = int(_o)                    # 9216


def _octa_nlon():
    half = NLAT // 2
    north = np.array([4 * (i + 1) + 16 for i in range(half)], dtype=np.int64)
    return np.concatenate([north, north[::-1]])


def _assign_groups():
    """Per-core list of NG north rings (group g -> one ring, padded to
    GSIZES[g] chunk slots)."""
    nlon = _octa_nlon()
    nch = np.ceil(nlon[:96] / CHUNK).astype(int)
    cls = {c: sorted(np.where(nch == c)[0].tolist()) for c in (1, 2, 3, 4)}
    assert [len(cls[c]) for c in (1, 2, 3, 4)] == [28, 32, 32, 4]
    c1, c2, c3, c4 = cls[1][:], cls[2][:], cls[3][:], cls[4][:]
    cores = []
    for c in range(NCORES):
        g4 = c4.pop() if c < 4 else c2.pop()
        g3s = [c3.pop() for _ in range(4)]
        g2s = [c2.pop() for _ in range(4)] if c < 4 else \
              [c2.pop() for _ in range(3)] + [c1.pop()]
        g1s = [c1.pop() for _ in range(3)]
        cores.append([g4] + g3s + g2s + g1s)
    assert not c1 and not c2 and not c3 and not c4
    return cores, nlon


def _build_core(rings, nlon, offs, x, E_re, E_im, PwT):
    xe = np.zeros((CHUNK, NSLOT, 2, BF), np.float16)
    ee = np.zeros((CHUNK, NSLOT, 2 * MMAX), np.float16)
    pwc = np.zeros((NROWS, MMAX, LMAX), np.float32)
    for g in range(NG):
        r = rings[g]
        rs = NLAT - 1 - r
        nl = int(nlon[r])
        for t in range(GSIZES[g]):
            j0 = t * CHUNK
            if j0 >= nl:
                continue
            s = int(GOFF[g]) + t
            jlen = min(CHUNK, nl - j0)
            xe[:jlen, s, 0, :] = x[:, offs[r] + j0: offs[r] + j0 + jlen].T
            xe[:jlen, s, 1, :] = x[:, offs[rs] + j0: offs[rs] + j0 + jlen].T
            elen = min(CHUNK, MAX_NLON - j0)
            ee[:elen, s, 0:MMAX] = E_re[r, j0:j0 + elen, :]
            ee[:elen, s, MMAX:] = E_im[r, j0:j0 + elen, :]
        pwc[2 * g] = PwT[:, r, :]          # [m, l]
        pwc[2 * g + 1] = PwT[:, rs, :]
    pwd = np.zeros((4 * NROWS, PWDW), np.float16)
    for q in range(32):
        grp, qi = divmod(q, 4)
        L = LLEN(16 * grp)
        lb = 16 * grp
        o = int(QD_OFF[q])
        for b in range(4):
            m = 16 * grp + qi + 4 * b
            pwd[NROWS * b:NROWS * (b + 1), o + b * L:o + (b + 1) * L] = \
                pwc[:, m, lb:]
    return (np.ascontiguousarray(xe.reshape(CHUNK, NSLOT * 256)),
            np.ascontiguousarray(ee.reshape(CHUNK, NSLOT * 256)),
            pwd)


def _build_bass():
    import concourse.mybir as mybir
    from concourse import bacc, tile

    dt = mybir.dt
    nc = bacc.Bacc()

    xe_d = nc.dram_tensor("xe", [CHUNK, NSLOT * 256], dt.float16,
                          kind="ExternalInput")
    ee_d = nc.dram_tensor("ee", [CHUNK, NSLOT * 256], dt.float16,
                          kind="ExternalInput")
    pwd_d = nc.dram_tensor("pwd", [4 * NROWS, PWDW], dt.float16,
                           kind="ExternalInput")
    outp_d = nc.dram_tensor("outp", [128, OUTW], dt.float16,
                            kind="ExternalOutput")
    gdram = nc.dram_tensor("gdram", [NROWS, MMAX * 256], dt.float16)

    with tile.TileContext(nc) as tc:
        with (
            tc.tile_pool(name="inp", bufs=1) as in_pool,
            tc.tile_pool(name="gsl", bufs=4) as gsl_pool,
            tc.tile_pool(name="gsb", bufs=3) as gsb_pool,
            tc.tile_pool(name="osb", bufs=3) as osb_pool,
            tc.tile_pool(name="ps1a", bufs=2, space="PSUM") as ps1a,
            tc.tile_pool(name="ps1b", bufs=2, space="PSUM") as ps1b,
            tc.tile_pool(name="ps2", bufs=4, space="PSUM") as ps2,
        ):
            xe = in_pool.tile([CHUNK, NSLOT * 256], dt.float16, tag="xe")
            ee = in_pool.tile([CHUNK, NSLOT * 256], dt.float16, tag="ee")
            pwd = in_pool.tile([4 * NROWS, PWDW], dt.float16, tag="pwd")

            # loads: xe on sync, ee on gpsimd (scalar stays copy-only in
            # phase 1); pwd late on gpsimd (phase-2 input). First chunk is
            # small so group-0 matmuls start early.
            GBL = [0, 4, 9, 15, 21, 27]
            for g in range(5):
                c0, c1 = GBL[g] * 256, GBL[g + 1] * 256
                nc.sync.dma_start(out=xe[:, c0:c1], in_=xe_d[:, c0:c1])
                nc.gpsimd.dma_start(out=ee[:, c0:c1], in_=ee_d[:, c0:c1])
            nc.gpsimd.dma_start(out=pwd[:], in_=pwd_d[:])

            cp_engines = [nc.scalar, nc.vector]

            def cp(idx, out, in_):
                e = cp_engines[idx % 2]
                if e is nc.scalar:
                    e.copy(out, in_)
                else:
                    e.tensor_copy(out, in_)

            # ---- phase 1: 12 ring-groups, psum accumulation over chunks ----
            ci = 0
            for g in range(NG):
                sz = GSIZES[g]
                # separate banks for the re / im accumulation chains (one
                # psum zero-region cannot host two pending groups)
                gre = ps1a.tile([MMAX, 512], dt.float32, tag="gre")
                gim = ps1b.tile([MMAX, 512], dt.float32, tag="gim")
                for t in range(sz):
                    s = int(GOFF[g]) + t
                    rhs = xe[:, s * 256:(s + 1) * 256]
                    st, sp = (t == 0), (t == sz - 1)
                    nc.tensor.matmul(gre[:, 0:256],
                                     ee[:, s * 256:s * 256 + 128],
                                     rhs, start=st, stop=sp)
                    nc.tensor.matmul(gim[:, 0:256],
                                     ee[:, s * 256 + 128:(s + 1) * 256],
                                     rhs, start=st, stop=sp)
                gsl = gsl_pool.tile([MMAX, 512], dt.float16, tag="gsl")
                # [re_n|re_s] + [im_n|im_s] -> ring-major [re_n|im_n|re_s|im_s]
                cp(ci + 0, gsl[:, 0:128], gre[:, 0:128])
                cp(ci + 1, gsl[:, 128:256], gim[:, 0:128])
                cp(ci + 2, gsl[:, 256:384], gre[:, 128:256])
                cp(ci + 3, gsl[:, 384:512], gim[:, 128:256])
                ci += 4
                # single DMA for both ring rows: dst [m, (k, 256)] view
                dst = gdram[2 * g:2 * g + 2].rearrange(
                    "k (m c) -> m k c", m=MMAX)
                nc.sync.dma_start(out=dst, in_=gsl[:])

            # ---- phase 2: 8 m-groups x 4 quads, one osb + out DMA per grp ----
            for grp in range(8):
                L = 128 - 16 * grp
                mg = 16 * grp
                # bands packed contiguously: row 24*b + r <- (ring r, band b)
                gsb = gsb_pool.tile([4 * NROWS, 1024], dt.float16, tag="gsb")
                src = gdram[:, mg * 256:(mg + 16) * 256]
                src = src.rearrange("r (b c) -> r b c", b=4).transpose([1, 0, 2])
                nc.scalar.dma_start(out=gsb[:], in_=src)
                gbase = int(QUAD_OFF[4 * grp])
                osb = osb_pool.tile([128, 32 * L], dt.float16, tag="osb")
                per_bank = max(1, 512 // (8 * L))
                po = None
                if L > 64:
                    # quad re / im each need their own psum bank (4L > 256)
                    for qi in range(4):
                        q = 4 * grp + qi
                        do = int(QD_OFF[q])
                        prhs = pwd[:, do:do + 4 * L]
                        qo = int(QUAD_OFF[q]) - gbase
                        for h in range(2):   # 0: re, 1: im
                            po = ps2.tile([128, 512], dt.float32, tag="po")
                            nc.tensor.matmul(
                                po[:, 0:4 * L],
                                gsb[:, qi * 256 + h * 128:qi * 256 + h * 128 + 128],
                                prhs, start=True, stop=True)
                            cp(ci, osb[:, qo + h * 4 * L:qo + (h + 1) * 4 * L],
                               po[:, 0:4 * L])
                            ci += 1
                else:
                    # whole quads (re+im = 8L <= 512) pack into banks
                    wb = per_bank * 8 * L
                    for qi in range(4):
                        q = 4 * grp + qi
                        do = int(QD_OFF[q])
                        prhs = pwd[:, do:do + 4 * L]
                        bq = qi % per_bank
                        if bq == 0:
                            po = ps2.tile([128, 512], dt.float32, tag="po")
                        c0 = bq * 8 * L
                        nc.tensor.matmul(po[:, c0:c0 + 4 * L],
                                         gsb[:, qi * 256:qi * 256 + 128],
                                         prhs, start=True, stop=True)
                        nc.tensor.matmul(po[:, c0 + 4 * L:c0 + 8 * L],
                                         gsb[:, qi * 256 + 128:(qi + 1) * 256],
                                         prhs, start=True, stop=True)
                        if bq == per_bank - 1:
                            qo = int(QUAD_OFF[q - per_bank + 1]) - gbase
                            cp(ci, osb[:, qo:qo + wb], po[:, 0:wb])
                            ci += 1
                we = nc.gpsimd if grp % 2 == 0 else nc.sync
                we.dma_start(out=outp_d[:, gbase:gbase + 32 * L], in_=osb[:])

    nc.compile()
    return nc


_CACHE = {}


def _get_compiled():
    if "nc" not in _CACHE:
        _CACHE["nc"] = _build_bass()
    return _CACHE["nc"]


def kernel(data, Pw, E_re, E_im, pad_idx):
    from concourse import bass_utils

    data = np.asarray(data)
    Pw = np.asarray(Pw, dtype=np.float32)
    E_re = np.asarray(E_re, dtype=np.float32)
    E_im = np.asarray(E_im, dtype=np.float32)

    cores, nlon = _assign_groups()
    offs = np.concatenate([[0], np.cumsum(nlon)[:-1]])
    x = np.ascontiguousarray(
        np.transpose(data, (0, 1, 3, 2)).reshape(BF, NPTS).astype(np.float32))
    PwT = np.ascontiguousarray(np.transpose(Pw, (1, 2, 0)))  # [m, n, l]

    in_maps = []
    for c in range(NCORES):
        xe, ee, pwd = _build_core(cores[c], nlon, offs, x, E_re, E_im, PwT)
        in_maps.append({"xe": xe, "ee": ee, "pwd": pwd})

    nc = _get_compiled()
    res = bass_utils.run_bass_kernel_spmd(nc, in_maps, list(range(NCORES)))
    _CACHE["last_results"] = res

    total = np.zeros((BF, OUTW), np.float64)
    for r in res.results:
        total += r["outp"].astype(np.float64)
    total = total.astype(np.float32)

    cc = np.zeros((LMAX, MMAX, BF), np.complex64)
    for q in range(32):
        grp, qi = divmod(q, 4)
        L = LLEN(16 * grp)
        lb = 16 * grp
        o = int(QUAD_OFF[q])
        for b in range(4):
            m = 16 * grp + qi + 4 * b
            re = total[:, o + b * L:o + (b + 1) * L]
            im = total[:, o + 4 * L + b * L:o + 4 * L + (b + 1) * L]
            cc[lb:, m, :] = (re + 1j * im).T
    cc = cc.reshape(LMAX, MMAX, B, V)
    out = np.transpose(cc, (2, 0, 1, 3))[:, None]
    return out.astype(np.complex64)


# revision 23
# speedup vs baseline: 3.7265x; 1.0773x over previous
"""Octahedral SHT on 8 NeuronCores (Bass/Tile) — v2.

v1 -> v2: per-ring PSUM accumulation (chunks of one ring accumulate on-chip
before the DRAM bounce) and 4-way m-packing in phase 2.

Sharding: 204 north DFT chunks -> 8 cores x 27 slots, organized as 12
ring-groups per core with the uniform size pattern [4,3,3,3,3,2,2,2,2,1,1,1]
(27 slots). Ring classes fit exactly: 4-groups take the 4 four-chunk rings +
4 two-chunk rings (padded), 3-groups the 32 three-chunk rings, 2-groups
28 two-chunk + 4 one-chunk rings, 1-groups 24 one-chunk rings. Each slot
also carries the mirrored south ring's chunk (identical DFT matrix E since
nlon is north/south symmetric), halving E traffic and PE weight loads.

Phase 1 (per group): psum[m, 512] = [re_n|re_s | im_n|im_s] accumulated
over the group's chunks (2 matmuls per slot, start on first chunk / stop on
last). Evacuate fp32->fp16 with 4 copies reordering to ring-major rows,
DMA 2 rows to gdram [24 rows, 128 m, 256] (row = ring: [re|im] per m).
Phase 2 (per 16-m group, per quad qi<4): the quad covers m = 16g+qi+4b for
bands b=0..3; gsb [128, 1024]: band b rows [32b, 32b+24) <- gdram m-columns
(contiguous). 2 matmuls per quad, K=128: lhsT = G_re/G_im [128, 128 bev],
rhs = block-diag pw [128, 4L] (band b rows -> pw_m(b) cols [bL,(b+1)L));
zero pw rows kill garbage lhsT rows. Only l >= 16*(m//16) is computed
(coeffs with l < m are structurally zero). Output fp16 [128 bev, 18432];
host sums the 8 partials and unpacks the triangle.
"""
import numpy as np

NLAT, LMAX, MMAX = 192, 128, 128
B, V = 2, 64
BF = B * V
NCORES = 8
CHUNK = 128
GSIZES = [4, 3, 3, 3, 3, 2, 2, 2, 2, 1, 1, 1]
NG = len(GSIZES)                  # 12 ring-groups per core
NSLOT = sum(GSIZES)               # 27
NROWS = 2 * NG                    # 24 G rows per core
GOFF = np.concatenate([[0], np.cumsum(GSIZES)]).astype(np.int64)
MAX_NLON = 400
NPTS = 40320


def LB(m):
    return 16 * (m // 16)


def LLEN(m):
    return LMAX - LB(m)


# quad q = 4*grp + qi -> m's { 16*grp + qi + 4*b : b in 0..3 }, L = 128-16*grp
QUAD_OFF = np.zeros(32, np.int64)
_o = 0
for _q in range(32):
    QUAD_OFF[_q] = _o
    _o += 8 * LLEN(16 * (_q // 4))
OUTW = int(_o)                    # 18432

QD_OFF = np.zeros(32, np.int64)
_o = 0
for _q in range(32):
    QD_OFF[_q] = _o
    _o += 4 * LLEN(16 * (_q // 4))
PWDW = int(_o)                    # 9216


def _octa_nlon():
    half = NLAT // 2
    north = np.array([4 * (i + 1) + 16 for i in range(half)], dtype=np.int64)
    return np.concatenate([north, north[::-1]])


def _assign_groups():
    """Per-core list of NG north rings (group g -> one ring, padded to
    GSIZES[g] chunk slots)."""
    nlon = _octa_nlon()
    nch = np.ceil(nlon[:96] / CHUNK).astype(int)
    cls = {c: sorted(np.where(nch == c)[0].tolist()) for c in (1, 2, 3, 4)}
    assert [len(cls[c]) for c in (1, 2, 3, 4)] == [28, 32, 32, 4]
    c1, c2, c3, c4 = cls[1][:], cls[2][:], cls[3][:], cls[4][:]
    cores = []
    for c in range(NCORES):
        g4 = c4.pop() if c < 4 else c2.pop()
        g3s = [c3.pop() for _ in range(4)]
        g2s = [c2.pop() for _ in range(4)] if c < 4 else \
              [c2.pop() for _ in range(3)] + [c1.pop()]
        g1s = [c1.pop() for _ in range(3)]
        cores.append([g4] + g3s + g2s + g1s)
    assert not c1 and not c2 and not c3 and not c4
    return cores, nlon


def _build_core(rings, nlon, offs, x, E_re, E_im, PwT):
    xe = np.zeros((CHUNK, NSLOT, 2, BF), np.float16)
    ee = np.zeros((CHUNK, NSLOT, 2 * MMAX), np.float16)
    pwc = np.zeros((NROWS, MMAX, LMAX), np.float32)
    for g in range(NG):
        r = rings[g]
        rs = NLAT - 1 - r
        nl = int(nlon[r])
        for t in range(GSIZES[g]):
            j0 = t * CHUNK
            if j0 >= nl:
                continue
            s = int(GOFF[g]) + t
            jlen = min(CHUNK, nl - j0)
            xe[:jlen, s, 0, :] = x[:, offs[r] + j0: offs[r] + j0 + jlen].T
            xe[:jlen, s, 1, :] = x[:, offs[rs] + j0: offs[rs] + j0 + jlen].T
            elen = min(CHUNK, MAX_NLON - j0)
            ee[:elen, s, 0:MMAX] = E_re[r, j0:j0 + elen, :]
            ee[:elen, s, MMAX:] = E_im[r, j0:j0 + elen, :]
        pwc[2 * g] = PwT[:, r, :]          # [m, l]
        pwc[2 * g + 1] = PwT[:, rs, :]
    # pwd row 4*r + b pairs with gsb row (ring r, band b)
    pwd = np.zeros((4 * NROWS, PWDW), np.float16)
    for q in range(32):
        grp, qi = divmod(q, 4)
        L = LLEN(16 * grp)
        lb = 16 * grp
        o = int(QD_OFF[q])
        for b in range(4):
            m = 16 * grp + qi + 4 * b
            pwd[4 * np.arange(NROWS) + b, o + b * L:o + (b + 1) * L] = \
                pwc[:, m, lb:]
    return (np.ascontiguousarray(xe.reshape(CHUNK, NSLOT * 256)),
            np.ascontiguousarray(ee.reshape(CHUNK, NSLOT * 256)),
            pwd)


def _build_bass():
    import concourse.mybir as mybir
    from concourse import bacc, tile

    dt = mybir.dt
    nc = bacc.Bacc()

    xe_d = nc.dram_tensor("xe", [CHUNK, NSLOT * 256], dt.float16,
                          kind="ExternalInput")
    ee_d = nc.dram_tensor("ee", [CHUNK, NSLOT * 256], dt.float16,
                          kind="ExternalInput")
    pwd_d = nc.dram_tensor("pwd", [4 * NROWS, PWDW], dt.float16,
                           kind="ExternalInput")
    outp_d = nc.dram_tensor("outp", [128, OUTW], dt.float16,
                            kind="ExternalOutput")
    gdram = nc.dram_tensor("gdram", [NROWS, MMAX * 256], dt.float16)

    with tile.TileContext(nc) as tc:
        with (
            tc.tile_pool(name="inp", bufs=1) as in_pool,
            tc.tile_pool(name="gsl", bufs=4) as gsl_pool,
            tc.tile_pool(name="gsb", bufs=3) as gsb_pool,
            tc.tile_pool(name="osb", bufs=3) as osb_pool,
            tc.tile_pool(name="ps1a", bufs=2, space="PSUM") as ps1a,
            tc.tile_pool(name="ps1b", bufs=2, space="PSUM") as ps1b,
            tc.tile_pool(name="ps2", bufs=4, space="PSUM") as ps2,
        ):
            xe = in_pool.tile([CHUNK, NSLOT * 256], dt.float16, tag="xe")
            ee = in_pool.tile([CHUNK, NSLOT * 256], dt.float16, tag="ee")
            pwd = in_pool.tile([4 * NROWS, PWDW], dt.float16, tag="pwd")

            # loads: xe on sync, ee on gpsimd (scalar stays copy-only in
            # phase 1); pwd late on gpsimd (phase-2 input). First chunk is
            # small so group-0 matmuls start early.
            GBL = [0, 4, 9, 15, 21, 27]
            for g in range(5):
                c0, c1 = GBL[g] * 256, GBL[g + 1] * 256
                nc.sync.dma_start(out=xe[:, c0:c1], in_=xe_d[:, c0:c1])
                nc.gpsimd.dma_start(out=ee[:, c0:c1], in_=ee_d[:, c0:c1])
            nc.gpsimd.dma_start(out=pwd[:], in_=pwd_d[:])

            cp_engines = [nc.scalar, nc.vector]

            def cp(idx, out, in_):
                e = cp_engines[idx % 2]
                if e is nc.scalar:
                    e.copy(out, in_)
                else:
                    e.tensor_copy(out, in_)

            # ---- phase 1: 12 ring-groups, psum accumulation over chunks ----
            ci = 0
            for g in range(NG):
                sz = GSIZES[g]
                # separate banks for the re / im accumulation chains (one
                # psum zero-region cannot host two pending groups)
                gre = ps1a.tile([MMAX, 512], dt.float32, tag="gre")
                gim = ps1b.tile([MMAX, 512], dt.float32, tag="gim")
                for t in range(sz):
                    s = int(GOFF[g]) + t
                    rhs = xe[:, s * 256:(s + 1) * 256]
                    st, sp = (t == 0), (t == sz - 1)
                    nc.tensor.matmul(gre[:, 0:256],
                                     ee[:, s * 256:s * 256 + 128],
                                     rhs, start=st, stop=sp)
                    nc.tensor.matmul(gim[:, 0:256],
                                     ee[:, s * 256 + 128:(s + 1) * 256],
                                     rhs, start=st, stop=sp)
                gsl = gsl_pool.tile([MMAX, 512], dt.float16, tag="gsl")
                # [re_n|re_s] + [im_n|im_s] -> ring-major [re_n|im_n|re_s|im_s]
                cp(ci + 0, gsl[:, 0:128], gre[:, 0:128])
                cp(ci + 1, gsl[:, 128:256], gim[:, 0:128])
                cp(ci + 2, gsl[:, 256:384], gre[:, 128:256])
                cp(ci + 3, gsl[:, 384:512], gim[:, 128:256])
                ci += 4
                # single DMA for both ring rows: dst [m, (k, 256)] view
                dst = gdram[2 * g:2 * g + 2].rearrange(
                    "k (m c) -> m k c", m=MMAX)
                nc.sync.dma_start(out=dst, in_=gsl[:])

            # ---- phase 2: 8 m-groups x 4 quads, one osb + out DMA per grp ----
            for grp in range(8):
                L = 128 - 16 * grp
                mg = 16 * grp
                # gsb row 4*r + b <- gdram[r, m-block mg+4b+qi]: the 16
                # m-blocks per ring are contiguous, so this is a straight
                # [24, 8KB] -> [96, 2KB] copy
                gsb = gsb_pool.tile([4 * NROWS, 1024], dt.float16, tag="gsb")
                nc.scalar.dma_start(out=gsb[:],
                                    in_=gdram[:, mg * 256:(mg + 16) * 256])
                gbase = int(QUAD_OFF[4 * grp])
                osb = osb_pool.tile([128, 32 * L], dt.float16, tag="osb")
                per_bank = max(1, 512 // (8 * L))
                po = None
                if L > 64:
                    # quad re / im each need their own psum bank (4L > 256)
                    for qi in range(4):
                        q = 4 * grp + qi
                        do = int(QD_OFF[q])
                        prhs = pwd[:, do:do + 4 * L]
                        qo = int(QUAD_OFF[q]) - gbase
                        for h in range(2):   # 0: re, 1: im
                            po = ps2.tile([128, 512], dt.float32, tag="po")
                            nc.tensor.matmul(
                                po[:, 0:4 * L],
                                gsb[:, qi * 256 + h * 128:qi * 256 + h * 128 + 128],
                                prhs, start=True, stop=True)
                            cp(ci, osb[:, qo + h * 4 * L:qo + (h + 1) * 4 * L],
                               po[:, 0:4 * L])
                            ci += 1
                else:
                    # whole quads (re+im = 8L <= 512) pack into banks
                    wb = per_bank * 8 * L
                    for qi in range(4):
                        q = 4 * grp + qi
                        do = int(QD_OFF[q])
                        prhs = pwd[:, do:do + 4 * L]
                        bq = qi % per_bank
                        if bq == 0:
                            po = ps2.tile([128, 512], dt.float32, tag="po")
                        c0 = bq * 8 * L
                        nc.tensor.matmul(po[:, c0:c0 + 4 * L],
                                         gsb[:, qi * 256:qi * 256 + 128],
                                         prhs, start=True, stop=True)
                        nc.tensor.matmul(po[:, c0 + 4 * L:c0 + 8 * L],
                                         gsb[:, qi * 256 + 128:(qi + 1) * 256],
                                         prhs, start=True, stop=True)
                        if bq == per_bank - 1:
                            qo = int(QUAD_OFF[q - per_bank + 1]) - gbase
                            cp(ci, osb[:, qo:qo + wb], po[:, 0:wb])
                            ci += 1
                we = nc.gpsimd if grp % 2 == 0 else nc.scalar
                we.dma_start(out=outp_d[:, gbase:gbase + 32 * L], in_=osb[:])

    nc.compile()
    return nc


_CACHE = {}


def _get_compiled():
    if "nc" not in _CACHE:
        _CACHE["nc"] = _build_bass()
    return _CACHE["nc"]


def kernel(data, Pw, E_re, E_im, pad_idx):
    from concourse import bass_utils

    data = np.asarray(data)
    Pw = np.asarray(Pw, dtype=np.float32)
    E_re = np.asarray(E_re, dtype=np.float32)
    E_im = np.asarray(E_im, dtype=np.float32)

    cores, nlon = _assign_groups()
    offs = np.concatenate([[0], np.cumsum(nlon)[:-1]])
    x = np.ascontiguousarray(
        np.transpose(data, (0, 1, 3, 2)).reshape(BF, NPTS).astype(np.float32))
    PwT = np.ascontiguousarray(np.transpose(Pw, (1, 2, 0)))  # [m, n, l]

    in_maps = []
    for c in range(NCORES):
        xe, ee, pwd = _build_core(cores[c], nlon, offs, x, E_re, E_im, PwT)
        in_maps.append({"xe": xe, "ee": ee, "pwd": pwd})

    nc = _get_compiled()
    res = bass_utils.run_bass_kernel_spmd(nc, in_maps, list(range(NCORES)))
    _CACHE["last_results"] = res

    total = np.zeros((BF, OUTW), np.float64)
    for r in res.results:
        total += r["outp"].astype(np.float64)
    total = total.astype(np.float32)

    cc = np.zeros((LMAX, MMAX, BF), np.complex64)
    for q in range(32):
        grp, qi = divmod(q, 4)
        L = LLEN(16 * grp)
        lb = 16 * grp
        o = int(QUAD_OFF[q])
        for b in range(4):
            m = 16 * grp + qi + 4 * b
            re = total[:, o + b * L:o + (b + 1) * L]
            im = total[:, o + 4 * L + b * L:o + 4 * L + (b + 1) * L]
            cc[lb:, m, :] = (re + 1j * im).T
    cc = cc.reshape(LMAX, MMAX, B, V)
    out = np.transpose(cc, (2, 0, 1, 3))[:, None]
    return out.astype(np.complex64)


# revision 26
# speedup vs baseline: 4.3438x; 1.1656x over previous
"""Octahedral SHT on 8 NeuronCores (Bass/Tile) — v2.

v1 -> v2: per-ring PSUM accumulation (chunks of one ring accumulate on-chip
before the DRAM bounce) and 4-way m-packing in phase 2.

Sharding: 204 north DFT chunks -> 8 cores x 27 slots, organized as 12
ring-groups per core with the uniform size pattern [4,3,3,3,3,2,2,2,2,1,1,1]
(27 slots). Ring classes fit exactly: 4-groups take the 4 four-chunk rings +
4 two-chunk rings (padded), 3-groups the 32 three-chunk rings, 2-groups
28 two-chunk + 4 one-chunk rings, 1-groups 24 one-chunk rings. Each slot
also carries the mirrored south ring's chunk (identical DFT matrix E since
nlon is north/south symmetric), halving E traffic and PE weight loads.

Phase 1 (per group): psum[m, 512] = [re_n|re_s | im_n|im_s] accumulated
over the group's chunks (2 matmuls per slot, start on first chunk / stop on
last). Evacuate fp32->fp16 with 4 copies reordering to ring-major rows,
DMA 2 rows to gdram [24 rows, 128 m, 256] (row = ring: [re|im] per m).
Phase 2 (per 16-m group, per quad qi<4): the quad covers m = 16g+qi+4b for
bands b=0..3; gsb [128, 1024]: band b rows [32b, 32b+24) <- gdram m-columns
(contiguous). 2 matmuls per quad, K=128: lhsT = G_re/G_im [128, 128 bev],
rhs = block-diag pw [128, 4L] (band b rows -> pw_m(b) cols [bL,(b+1)L));
zero pw rows kill garbage lhsT rows. Only l >= 16*(m//16) is computed
(coeffs with l < m are structurally zero). Output fp16 [128 bev, 18432];
host sums the 8 partials and unpacks the triangle.
"""
import numpy as np

NLAT, LMAX, MMAX = 192, 128, 128
B, V = 2, 64
BF = B * V
NCORES = 8
CHUNK = 128
GSIZES = [4, 3, 3, 3, 3, 2, 2, 2, 2, 1, 1, 1]
NG = len(GSIZES)                  # 12 ring-groups per core
NSLOT = sum(GSIZES)               # 27
NROWS = 2 * NG                    # 24 G rows per core
GOFF = np.concatenate([[0], np.cumsum(GSIZES)]).astype(np.int64)
MAX_NLON = 400
NPTS = 40320


def LB(m):
    return 16 * (m // 16)


def LLEN(m):
    return LMAX - LB(m)


# quad q = 4*grp + qi -> m's { 16*grp + qi + 4*b : b in 0..3 }, L = 128-16*grp
QUAD_OFF = np.zeros(32, np.int64)
_o = 0
for _q in range(32):
    QUAD_OFF[_q] = _o
    _o += 8 * LLEN(16 * (_q // 4))
OUTW = int(_o)                    # 18432

QD_OFF = np.zeros(32, np.int64)
_o = 0
for _q in range(32):
    QD_OFF[_q] = _o
    _o += 4 * LLEN(16 * (_q // 4))
PWDW = int(_o)                    # 9216


def _octa_nlon():
    half = NLAT // 2
    north = np.array([4 * (i + 1) + 16 for i in range(half)], dtype=np.int64)
    return np.concatenate([north, north[::-1]])


def _assign_groups():
    """Per-core list of NG north rings (group g -> one ring, padded to
    GSIZES[g] chunk slots)."""
    nlon = _octa_nlon()
    nch = np.ceil(nlon[:96] / CHUNK).astype(int)
    cls = {c: sorted(np.where(nch == c)[0].tolist()) for c in (1, 2, 3, 4)}
    assert [len(cls[c]) for c in (1, 2, 3, 4)] == [28, 32, 32, 4]
    c1, c2, c3, c4 = cls[1][:], cls[2][:], cls[3][:], cls[4][:]
    cores = []
    for c in range(NCORES):
        g4 = c4.pop() if c < 4 else c2.pop()
        g3s = [c3.pop() for _ in range(4)]
        g2s = [c2.pop() for _ in range(4)] if c < 4 else \
              [c2.pop() for _ in range(3)] + [c1.pop()]
        g1s = [c1.pop() for _ in range(3)]
        cores.append([g4] + g3s + g2s + g1s)
    assert not c1 and not c2 and not c3 and not c4
    return cores, nlon


def _build_core(rings, nlon, offs, x, E_re, E_im, PwT):
    xe = np.zeros((CHUNK, NSLOT, 2, BF), np.float16)
    ee = np.zeros((CHUNK, NSLOT, 2 * MMAX), np.float16)
    pwc = np.zeros((NROWS, MMAX, LMAX), np.float32)
    for g in range(NG):
        r = rings[g]
        rs = NLAT - 1 - r
        nl = int(nlon[r])
        for t in range(GSIZES[g]):
            j0 = t * CHUNK
            if j0 >= nl:
                continue
            s = int(GOFF[g]) + t
            jlen = min(CHUNK, nl - j0)
            xe[:jlen, s, 0, :] = x[:, offs[r] + j0: offs[r] + j0 + jlen].T
            xe[:jlen, s, 1, :] = x[:, offs[rs] + j0: offs[rs] + j0 + jlen].T
            elen = min(CHUNK, MAX_NLON - j0)
            ee[:elen, s, 0:MMAX] = E_re[r, j0:j0 + elen, :]
            ee[:elen, s, MMAX:] = E_im[r, j0:j0 + elen, :]
        pwc[2 * g] = PwT[:, r, :]          # [m, l]
        pwc[2 * g + 1] = PwT[:, rs, :]
    # pwd row 4*r + b pairs with gsb row (ring r, band b)
    pwd = np.zeros((4 * NROWS, PWDW), np.float16)
    for q in range(32):
        grp, qi = divmod(q, 4)
        L = LLEN(16 * grp)
        lb = 16 * grp
        o = int(QD_OFF[q])
        for b in range(4):
            m = 16 * grp + qi + 4 * b
            pwd[4 * np.arange(NROWS) + b, o + b * L:o + (b + 1) * L] = \
                pwc[:, m, lb:]
    return (np.ascontiguousarray(xe.reshape(CHUNK, NSLOT * 256)),
            np.ascontiguousarray(ee.reshape(CHUNK, NSLOT * 256)),
            pwd)


def _build_bass():
    import concourse.mybir as mybir
    from concourse import bacc, tile

    dt = mybir.dt
    nc = bacc.Bacc()

    xe_d = nc.dram_tensor("xe", [CHUNK, NSLOT * 256], dt.float16,
                          kind="ExternalInput")
    ee_d = nc.dram_tensor("ee", [CHUNK, NSLOT * 256], dt.float16,
                          kind="ExternalInput")
    pwd_d = nc.dram_tensor("pwd", [4 * NROWS, PWDW], dt.float16,
                           kind="ExternalInput")
    outp_d = nc.dram_tensor("outp", [128, OUTW], dt.float16,
                            kind="ExternalOutput")
    gdram = nc.dram_tensor("gdram", [NROWS, MMAX * 256], dt.float16)

    with tile.TileContext(nc) as tc:
        with (
            tc.tile_pool(name="inp", bufs=1) as in_pool,
            tc.tile_pool(name="gsl", bufs=12) as gsl_pool,
            tc.tile_pool(name="gsb", bufs=4) as gsb_pool,
            tc.tile_pool(name="osb", bufs=3) as osb_pool,
            tc.tile_pool(name="ps1a", bufs=2, space="PSUM") as ps1a,
            tc.tile_pool(name="ps1b", bufs=2, space="PSUM") as ps1b,
            tc.tile_pool(name="ps2", bufs=4, space="PSUM") as ps2,
        ):
            xe = in_pool.tile([CHUNK, NSLOT * 256], dt.float16, tag="xe")
            ee = in_pool.tile([CHUNK, NSLOT * 256], dt.float16, tag="ee")
            pwd = in_pool.tile([4 * NROWS, PWDW], dt.float16, tag="pwd")

            # loads: xe on sync, ee on gpsimd (scalar stays copy-only in
            # phase 1); pwd late on gpsimd (phase-2 input). First chunk is
            # small so group-0 matmuls start early.
            GBL = [0, 4, 9, 15, 21, 27]
            for g in range(5):
                c0, c1 = GBL[g] * 256, GBL[g + 1] * 256
                xq = nc.sync if g % 2 == 0 else nc.scalar
                xq.dma_start(out=xe[:, c0:c1], in_=xe_d[:, c0:c1])
                nc.gpsimd.dma_start(out=ee[:, c0:c1], in_=ee_d[:, c0:c1])
            nc.gpsimd.dma_start(out=pwd[:], in_=pwd_d[:])

            cp_engines = [nc.scalar, nc.vector]

            def cp(idx, out, in_):
                e = cp_engines[idx % 2]
                if e is nc.scalar:
                    e.copy(out, in_)
                else:
                    e.tensor_copy(out, in_)

            # ---- phase 1: 12 ring-groups, psum accumulation over chunks ----
            ci = 0
            for g in range(NG):
                sz = GSIZES[g]
                # separate banks for the re / im accumulation chains (one
                # psum zero-region cannot host two pending groups)
                gre = ps1a.tile([MMAX, 512], dt.float32, tag="gre")
                gim = ps1b.tile([MMAX, 512], dt.float32, tag="gim")
                for t in range(sz):
                    s = int(GOFF[g]) + t
                    rhs = xe[:, s * 256:(s + 1) * 256]
                    st, sp = (t == 0), (t == sz - 1)
                    nc.tensor.matmul(gre[:, 0:256],
                                     ee[:, s * 256:s * 256 + 128],
                                     rhs, start=st, stop=sp)
                    nc.tensor.matmul(gim[:, 0:256],
                                     ee[:, s * 256 + 128:(s + 1) * 256],
                                     rhs, start=st, stop=sp)
                gsl = gsl_pool.tile([MMAX, 512], dt.float16, tag="gsl")
                # [re_n|re_s] + [im_n|im_s] -> ring-major [re_n|im_n|re_s|im_s]
                cp(ci + 0, gsl[:, 0:128], gre[:, 0:128])
                cp(ci + 1, gsl[:, 128:256], gim[:, 0:128])
                cp(ci + 2, gsl[:, 256:384], gre[:, 128:256])
                cp(ci + 3, gsl[:, 384:512], gim[:, 128:256])
                ci += 4
                # single DMA for both ring rows: dst [m, (k, 256)] view
                dst = gdram[2 * g:2 * g + 2].rearrange(
                    "k (m c) -> m k c", m=MMAX)
                nc.sync.dma_start(out=dst, in_=gsl[:])

            # ---- phase 2: 8 m-groups x 4 quads, one osb + out DMA per grp ----
            for grp in range(8):
                L = 128 - 16 * grp
                mg = 16 * grp
                # gsb row 4*r + b <- gdram[r, m-block mg+4b+qi]: the 16
                # m-blocks per ring are contiguous, so this is a straight
                # [24, 8KB] -> [96, 2KB] copy
                gsb = gsb_pool.tile([4 * NROWS, 1024], dt.float16, tag="gsb")
                nc.scalar.dma_start(out=gsb[:],
                                    in_=gdram[:, mg * 256:(mg + 16) * 256])
                gbase = int(QUAD_OFF[4 * grp])
                osb = osb_pool.tile([128, 32 * L], dt.float16, tag="osb")
                per_bank = max(1, 512 // (8 * L))
                po = None
                if L > 64:
                    # quad re / im each need their own psum bank (4L > 256)
                    for qi in range(4):
                        q = 4 * grp + qi
                        do = int(QD_OFF[q])
                        prhs = pwd[:, do:do + 4 * L]
                        qo = int(QUAD_OFF[q]) - gbase
                        for h in range(2):   # 0: re, 1: im
                            po = ps2.tile([128, 512], dt.float32, tag="po")
                            nc.tensor.matmul(
                                po[:, 0:4 * L],
                                gsb[:, qi * 256 + h * 128:qi * 256 + h * 128 + 128],
                                prhs, start=True, stop=True)
                            cp(ci, osb[:, qo + h * 4 * L:qo + (h + 1) * 4 * L],
                               po[:, 0:4 * L])
                            ci += 1
                else:
                    # whole quads (re+im = 8L <= 512) pack into banks
                    wb = per_bank * 8 * L
                    for qi in range(4):
                        q = 4 * grp + qi
                        do = int(QD_OFF[q])
                        prhs = pwd[:, do:do + 4 * L]
                        bq = qi % per_bank
                        if bq == 0:
                            po = ps2.tile([128, 512], dt.float32, tag="po")
                        c0 = bq * 8 * L
                        nc.tensor.matmul(po[:, c0:c0 + 4 * L],
                                         gsb[:, qi * 256:qi * 256 + 128],
                                         prhs, start=True, stop=True)
                        nc.tensor.matmul(po[:, c0 + 4 * L:c0 + 8 * L],
                                         gsb[:, qi * 256 + 128:(qi + 1) * 256],
                                         prhs, start=True, stop=True)
                        if bq == per_bank - 1:
                            qo = int(QUAD_OFF[q - per_bank + 1]) - gbase
                            cp(ci, osb[:, qo:qo + wb], po[:, 0:wb])
                            ci += 1
                we = nc.gpsimd if grp % 2 == 0 else nc.sync
                we.dma_start(out=outp_d[:, gbase:gbase + 32 * L], in_=osb[:])

    nc.compile()
    return nc


_CACHE = {}


def _get_compiled():
    if "nc" not in _CACHE:
        _CACHE["nc"] = _build_bass()
    return _CACHE["nc"]


def kernel(data, Pw, E_re, E_im, pad_idx):
    from concourse import bass_utils

    data = np.asarray(data)
    Pw = np.asarray(Pw, dtype=np.float32)
    E_re = np.asarray(E_re, dtype=np.float32)
    E_im = np.asarray(E_im, dtype=np.float32)

    cores, nlon = _assign_groups()
    offs = np.concatenate([[0], np.cumsum(nlon)[:-1]])
    x = np.ascontiguousarray(
        np.transpose(data, (0, 1, 3, 2)).reshape(BF, NPTS).astype(np.float32))
    PwT = np.ascontiguousarray(np.transpose(Pw, (1, 2, 0)))  # [m, n, l]

    in_maps = []
    for c in range(NCORES):
        xe, ee, pwd = _build_core(cores[c], nlon, offs, x, E_re, E_im, PwT)
        in_maps.append({"xe": xe, "ee": ee, "pwd": pwd})

    nc = _get_compiled()
    res = bass_utils.run_bass_kernel_spmd(nc, in_maps, list(range(NCORES)))
    _CACHE["last_results"] = res

    total = np.zeros((BF, OUTW), np.float64)
    for r in res.results:
        total += r["outp"].astype(np.float64)
    total = total.astype(np.float32)

    cc = np.zeros((LMAX, MMAX, BF), np.complex64)
    for q in range(32):
        grp, qi = divmod(q, 4)
        L = LLEN(16 * grp)
        lb = 16 * grp
        o = int(QUAD_OFF[q])
        for b in range(4):
            m = 16 * grp + qi + 4 * b
            re = total[:, o + b * L:o + (b + 1) * L]
            im = total[:, o + 4 * L + b * L:o + 4 * L + (b + 1) * L]
            cc[lb:, m, :] = (re + 1j * im).T
    cc = cc.reshape(LMAX, MMAX, B, V)
    out = np.transpose(cc, (2, 0, 1, 3))[:, None]
    return out.astype(np.complex64)


# revision 29
# speedup vs baseline: 4.5229x; 1.0412x over previous
"""Octahedral SHT on 8 NeuronCores (Bass/Tile) — v2.

v1 -> v2: per-ring PSUM accumulation (chunks of one ring accumulate on-chip
before the DRAM bounce) and 4-way m-packing in phase 2.

Sharding: 204 north DFT chunks -> 8 cores x 27 slots, organized as 12
ring-groups per core with the uniform size pattern [4,3,3,3,3,2,2,2,2,1,1,1]
(27 slots). Ring classes fit exactly: 4-groups take the 4 four-chunk rings +
4 two-chunk rings (padded), 3-groups the 32 three-chunk rings, 2-groups
28 two-chunk + 4 one-chunk rings, 1-groups 24 one-chunk rings. Each slot
also carries the mirrored south ring's chunk (identical DFT matrix E since
nlon is north/south symmetric), halving E traffic and PE weight loads.

Phase 1 (per group): psum[m, 512] = [re_n|re_s | im_n|im_s] accumulated
over the group's chunks (2 matmuls per slot, start on first chunk / stop on
last). Evacuate fp32->fp16 with 4 copies reordering to ring-major rows,
DMA 2 rows to gdram [24 rows, 128 m, 256] (row = ring: [re|im] per m).
Phase 2 (per 16-m group, per quad qi<4): the quad covers m = 16g+qi+4b for
bands b=0..3; gsb [128, 1024]: band b rows [32b, 32b+24) <- gdram m-columns
(contiguous). 2 matmuls per quad, K=128: lhsT = G_re/G_im [128, 128 bev],
rhs = block-diag pw [128, 4L] (band b rows -> pw_m(b) cols [bL,(b+1)L));
zero pw rows kill garbage lhsT rows. Only l >= 16*(m//16) is computed
(coeffs with l < m are structurally zero). Output fp16 [128 bev, 18432];
host sums the 8 partials and unpacks the triangle.
"""
import numpy as np

NLAT, LMAX, MMAX = 192, 128, 128
B, V = 2, 64
BF = B * V
NCORES = 8
CHUNK = 128
GSIZES = [4, 3, 3, 3, 3, 2, 2, 2, 2, 1, 1, 1]
NG = len(GSIZES)                  # 12 ring-groups per core
NSLOT = sum(GSIZES)               # 27
NROWS = 2 * NG                    # 24 G rows per core
GOFF = np.concatenate([[0], np.cumsum(GSIZES)]).astype(np.int64)
MAX_NLON = 400
NPTS = 40320


def LB(m):
    return 16 * (m // 16)


def LLEN(m):
    return LMAX - LB(m)


# quad q = 4*grp + qi -> m's { 16*grp + qi + 4*b : b in 0..3 }, L = 128-16*grp
QUAD_OFF = np.zeros(32, np.int64)
_o = 0
for _q in range(32):
    QUAD_OFF[_q] = _o
    _o += 8 * LLEN(16 * (_q // 4))
OUTW = int(_o)                    # 18432

QD_OFF = np.zeros(32, np.int64)
_o = 0
for _q in range(32):
    QD_OFF[_q] = _o
    _o += 4 * LLEN(16 * (_q // 4))
PWDW = int(_o)                    # 9216


def _octa_nlon():
    half = NLAT // 2
    north = np.array([4 * (i + 1) + 16 for i in range(half)], dtype=np.int64)
    return np.concatenate([north, north[::-1]])


def _assign_groups():
    """Per-core list of NG north rings (group g -> one ring, padded to
    GSIZES[g] chunk slots)."""
    nlon = _octa_nlon()
    nch = np.ceil(nlon[:96] / CHUNK).astype(int)
    cls = {c: sorted(np.where(nch == c)[0].tolist()) for c in (1, 2, 3, 4)}
    assert [len(cls[c]) for c in (1, 2, 3, 4)] == [28, 32, 32, 4]
    c1, c2, c3, c4 = cls[1][:], cls[2][:], cls[3][:], cls[4][:]
    cores = []
    for c in range(NCORES):
        g4 = c4.pop() if c < 4 else c2.pop()
        g3s = [c3.pop() for _ in range(4)]
        g2s = [c2.pop() for _ in range(4)] if c < 4 else \
              [c2.pop() for _ in range(3)] + [c1.pop()]
        g1s = [c1.pop() for _ in range(3)]
        cores.append([g4] + g3s + g2s + g1s)
    assert not c1 and not c2 and not c3 and not c4
    return cores, nlon


def _build_core(rings, nlon, offs, x, E_re, E_im, PwT):
    xe = np.zeros((CHUNK, NSLOT, 2, BF), np.float16)
    ee = np.zeros((CHUNK, NSLOT, 2 * MMAX), np.float16)
    pwc = np.zeros((NROWS, MMAX, LMAX), np.float32)
    for g in range(NG):
        r = rings[g]
        rs = NLAT - 1 - r
        nl = int(nlon[r])
        for t in range(GSIZES[g]):
            j0 = t * CHUNK
            if j0 >= nl:
                continue
            s = int(GOFF[g]) + t
            jlen = min(CHUNK, nl - j0)
            xe[:jlen, s, 0, :] = x[:, offs[r] + j0: offs[r] + j0 + jlen].T
            xe[:jlen, s, 1, :] = x[:, offs[rs] + j0: offs[rs] + j0 + jlen].T
            elen = min(CHUNK, MAX_NLON - j0)
            ee[:elen, s, 0:MMAX] = E_re[r, j0:j0 + elen, :]
            ee[:elen, s, MMAX:] = E_im[r, j0:j0 + elen, :]
        pwc[2 * g] = PwT[:, r, :]          # [m, l]
        pwc[2 * g + 1] = PwT[:, rs, :]
    # pwd row 4*r + b pairs with gsb row (ring r, band b)
    pwd = np.zeros((4 * NROWS, PWDW), np.float16)
    for q in range(32):
        grp, qi = divmod(q, 4)
        L = LLEN(16 * grp)
        lb = 16 * grp
        o = int(QD_OFF[q])
        for b in range(4):
            m = 16 * grp + qi + 4 * b
            pwd[4 * np.arange(NROWS) + b, o + b * L:o + (b + 1) * L] = \
                pwc[:, m, lb:]
    return (np.ascontiguousarray(xe.reshape(CHUNK, NSLOT * 256)),
            np.ascontiguousarray(ee.reshape(CHUNK, NSLOT * 256)),
            pwd)


def _build_bass():
    import concourse.mybir as mybir
    from concourse import bacc, tile

    dt = mybir.dt
    nc = bacc.Bacc()

    xe_d = nc.dram_tensor("xe", [CHUNK, NSLOT * 256], dt.float16,
                          kind="ExternalInput")
    ee_d = nc.dram_tensor("ee", [CHUNK, NSLOT * 256], dt.float16,
                          kind="ExternalInput")
    pwd_d = nc.dram_tensor("pwd", [4 * NROWS, PWDW], dt.float16,
                           kind="ExternalInput")
    outp_d = nc.dram_tensor("outp", [128, OUTW], dt.float16,
                            kind="ExternalOutput")
    gdram = nc.dram_tensor("gdram", [NROWS, MMAX * 256], dt.float16)

    with tile.TileContext(nc) as tc:
        with (
            tc.tile_pool(name="inp", bufs=1) as in_pool,
            tc.tile_pool(name="gsl", bufs=6) as gsl_pool,
            tc.tile_pool(name="gsb", bufs=8) as gsb_pool,
            tc.tile_pool(name="osb", bufs=3) as osb_pool,
            tc.tile_pool(name="ps1a", bufs=2, space="PSUM") as ps1a,
            tc.tile_pool(name="ps1b", bufs=2, space="PSUM") as ps1b,
            tc.tile_pool(name="ps2", bufs=4, space="PSUM") as ps2,
        ):
            xe = in_pool.tile([CHUNK, NSLOT * 256], dt.float16, tag="xe")
            ee = in_pool.tile([CHUNK, NSLOT * 256], dt.float16, tag="ee")
            pwd = in_pool.tile([4 * NROWS, PWDW], dt.float16, tag="pwd")

            # loads: xe on sync, ee on gpsimd (scalar stays copy-only in
            # phase 1); pwd late on gpsimd (phase-2 input). First chunk is
            # small so group-0 matmuls start early.
            GBL = [0, 4, 9, 15, 21, 27]
            for g in range(5):
                c0, c1 = GBL[g] * 256, GBL[g + 1] * 256
                xq = nc.sync if g % 2 == 0 else nc.scalar
                xq.dma_start(out=xe[:, c0:c1], in_=xe_d[:, c0:c1])
                nc.gpsimd.dma_start(out=ee[:, c0:c1], in_=ee_d[:, c0:c1])
            nc.gpsimd.dma_start(out=pwd[:], in_=pwd_d[:])

            cp_engines = [nc.scalar, nc.vector]

            def cp(idx, out, in_):
                e = cp_engines[idx % 2]
                if e is nc.scalar:
                    e.copy(out, in_)
                else:
                    e.tensor_copy(out, in_)

            # ---- phase 1: 12 ring-groups, psum accumulation over chunks ----
            ci = 0
            gsl = None
            for g in range(NG):
                sz = GSIZES[g]
                # separate banks for the re / im accumulation chains (one
                # psum zero-region cannot host two pending groups)
                gre = ps1a.tile([MMAX, 512], dt.float32, tag="gre")
                gim = ps1b.tile([MMAX, 512], dt.float32, tag="gim")
                for t in range(sz):
                    s = int(GOFF[g]) + t
                    rhs = xe[:, s * 256:(s + 1) * 256]
                    st, sp = (t == 0), (t == sz - 1)
                    nc.tensor.matmul(gre[:, 0:256],
                                     ee[:, s * 256:s * 256 + 128],
                                     rhs, start=st, stop=sp)
                    nc.tensor.matmul(gim[:, 0:256],
                                     ee[:, s * 256 + 128:(s + 1) * 256],
                                     rhs, start=st, stop=sp)
                if g % 2 == 0:
                    gsl = gsl_pool.tile([MMAX, 1024], dt.float16, tag="gsl")
                go = 512 * (g % 2)
                # [re_n|re_s] + [im_n|im_s] -> ring-major [re_n|im_n|re_s|im_s]
                cp(ci + 0, gsl[:, go + 0:go + 128], gre[:, 0:128])
                cp(ci + 1, gsl[:, go + 128:go + 256], gim[:, 0:128])
                cp(ci + 2, gsl[:, go + 256:go + 384], gre[:, 128:256])
                cp(ci + 3, gsl[:, go + 384:go + 512], gim[:, 128:256])
                ci += 4
                if g % 2 == 1:
                    # one DMA per group pair: 4 ring rows
                    dst = gdram[2 * g - 2:2 * g + 2].rearrange(
                        "k (m c) -> m k c", m=MMAX)
                    we = nc.sync if g % 4 == 1 else nc.gpsimd
                    we.dma_start(out=dst, in_=gsl[:])

            # ---- phase 2: 8 m-groups x 4 quads, one osb + out DMA per grp ----
            # prefetch all gsb blocks upfront (split over two queues).
            # gsb row 4*r + b <- gdram[r, m-block mg+4b+qi]: the 16 m-blocks
            # per ring are contiguous, so each is a straight [24, 8KB] copy
            gsbs = []
            for grp in range(8):
                gsb = gsb_pool.tile([4 * NROWS, 1024], dt.float16, tag="gsb")
                gq = nc.scalar if grp % 2 == 0 else nc.gpsimd
                gq.dma_start(out=gsb[:],
                             in_=gdram[:, grp * 16 * 256:(grp + 1) * 16 * 256])
                gsbs.append(gsb)
            for grp in range(8):
                L = 128 - 16 * grp
                gsb = gsbs[grp]
                gbase = int(QUAD_OFF[4 * grp])
                osb = osb_pool.tile([128, 32 * L], dt.float16, tag="osb")
                per_bank = max(1, 512 // (8 * L))
                po = None
                if L > 64:
                    # quad re / im each need their own psum bank (4L > 256)
                    for qi in range(4):
                        q = 4 * grp + qi
                        do = int(QD_OFF[q])
                        prhs = pwd[:, do:do + 4 * L]
                        qo = int(QUAD_OFF[q]) - gbase
                        for h in range(2):   # 0: re, 1: im
                            po = ps2.tile([128, 512], dt.float32, tag="po")
                            nc.tensor.matmul(
                                po[:, 0:4 * L],
                                gsb[:, qi * 256 + h * 128:qi * 256 + h * 128 + 128],
                                prhs, start=True, stop=True)
                            cp(ci, osb[:, qo + h * 4 * L:qo + (h + 1) * 4 * L],
                               po[:, 0:4 * L])
                            ci += 1
                else:
                    # whole quads (re+im = 8L <= 512) pack into banks
                    wb = per_bank * 8 * L
                    for qi in range(4):
                        q = 4 * grp + qi
                        do = int(QD_OFF[q])
                        prhs = pwd[:, do:do + 4 * L]
                        bq = qi % per_bank
                        if bq == 0:
                            po = ps2.tile([128, 512], dt.float32, tag="po")
                        c0 = bq * 8 * L
                        nc.tensor.matmul(po[:, c0:c0 + 4 * L],
                                         gsb[:, qi * 256:qi * 256 + 128],
                                         prhs, start=True, stop=True)
                        nc.tensor.matmul(po[:, c0 + 4 * L:c0 + 8 * L],
                                         gsb[:, qi * 256 + 128:(qi + 1) * 256],
                                         prhs, start=True, stop=True)
                        if bq == per_bank - 1:
                            qo = int(QUAD_OFF[q - per_bank + 1]) - gbase
                            cp(ci, osb[:, qo:qo + wb], po[:, 0:wb])
                            ci += 1
                we = nc.gpsimd if grp % 2 == 0 else nc.sync
                we.dma_start(out=outp_d[:, gbase:gbase + 32 * L], in_=osb[:])

    nc.compile()
    return nc


_CACHE = {}


def _get_compiled():
    if "nc" not in _CACHE:
        _CACHE["nc"] = _build_bass()
    return _CACHE["nc"]


def kernel(data, Pw, E_re, E_im, pad_idx):
    from concourse import bass_utils

    data = np.asarray(data)
    Pw = np.asarray(Pw, dtype=np.float32)
    E_re = np.asarray(E_re, dtype=np.float32)
    E_im = np.asarray(E_im, dtype=np.float32)

    cores, nlon = _assign_groups()
    offs = np.concatenate([[0], np.cumsum(nlon)[:-1]])
    x = np.ascontiguousarray(
        np.transpose(data, (0, 1, 3, 2)).reshape(BF, NPTS).astype(np.float32))
    PwT = np.ascontiguousarray(np.transpose(Pw, (1, 2, 0)))  # [m, n, l]

    in_maps = []
    for c in range(NCORES):
        xe, ee, pwd = _build_core(cores[c], nlon, offs, x, E_re, E_im, PwT)
        in_maps.append({"xe": xe, "ee": ee, "pwd": pwd})

    nc = _get_compiled()
    res = bass_utils.run_bass_kernel_spmd(nc, in_maps, list(range(NCORES)))
    _CACHE["last_results"] = res

    total = np.zeros((BF, OUTW), np.float64)
    for r in res.results:
        total += r["outp"].astype(np.float64)
    total = total.astype(np.float32)

    cc = np.zeros((LMAX, MMAX, BF), np.complex64)
    for q in range(32):
        grp, qi = divmod(q, 4)
        L = LLEN(16 * grp)
        lb = 16 * grp
        o = int(QUAD_OFF[q])
        for b in range(4):
            m = 16 * grp + qi + 4 * b
            re = total[:, o + b * L:o + (b + 1) * L]
            im = total[:, o + 4 * L + b * L:o + 4 * L + (b + 1) * L]
            cc[lb:, m, :] = (re + 1j * im).T
    cc = cc.reshape(LMAX, MMAX, B, V)
    out = np.transpose(cc, (2, 0, 1, 3))[:, None]
    return out.astype(np.complex64)
